# revision 1
# baseline (speedup 1.0000x reference)
"""Multi-head causal attention (B=2, T=2048, E=1024, H=16, D=64) on 8 trn2 cores.

Sharding: core c -> batch b = c // 4, head-group hg = c % 4 (4 heads each).
Per-core: QKV projections for its 4 heads, causal flash attention in
transposed-score layout (S^T[k,q]; softmax denominator folded into a
ones-augmented V matmul), row-parallel output projection producing a partial
[T, E] output. Host sums the 4 partials per batch and adds the bias.
"""
import sys
from contextlib import ExitStack

sys.path.insert(0, "/opt/trn_rl_repo")

import numpy as np

import concourse.bass as bass
import concourse.tile as tile
from concourse import bacc, mybir
from concourse.bass_utils import run_bass_kernel_spmd

F32 = mybir.dt.float32
F32R = mybir.dt.float32r
EXP = mybir.ActivationFunctionType.Exp

B, T, E, H = 2, 2048, 1024, 16
D = E // H              # 64
N_CORES = 8
GH = 4                  # heads per core
GE = GH * D             # 256 per-core projection width
SCALE = float(D) ** -0.5

TCH = 512               # projection t-chunk
NTCH = T // TCH         # 4
KC = 8                  # contraction chunks of 128 over E
QB = 512                # attention q-block
NQB = T // QB           # 4
KB = 128                # attention k-block


DEFAULT_OPTS = dict(
    interleave=False,    # head-interleaved emission (PE row-group packing) -- off: modeled slower
    s_bufs=2,            # S psum slots ([128,1024] = 2 banks each)
    y_in_s=False,
    o_bufs=3,
    p_bufs=6,
    x_bufs=10,
    v_bufs=3,
    y_split=True,        # Y psum as two single-bank [128,512] tiles
    y_bufs=1,
    y_last_in_s=True,    # final q-block Y tiles borrow the idle S slots
    recip_fast=False,    # approx recip (custom DVE) produced garbage on HW -- keep exact
    norm_splits_last=4,  # split the last q-block's normalize per q-tile
    l_bufs=6,
    on_bufs=6,
    yst_bufs=4,          # more Y staging slots pipeline the out-projection tail
)


def build_program(opts=None):
    o = dict(DEFAULT_OPTS)
    if opts:
        o.update(opts)
    nc = bacc.Bacc("TRN2", target_bir_lowering=False, debug=False, num_devices=N_CORES)

    xt_d = nc.dram_tensor("xt", [E, T], F32, kind="ExternalInput").ap()
    wqt_d = nc.dram_tensor("wqt", [E, GE], F32, kind="ExternalInput").ap()
    wkt_d = nc.dram_tensor("wkt", [E, GE], F32, kind="ExternalInput").ap()
    wvt_d = nc.dram_tensor("wvt", [E, GE], F32, kind="ExternalInput").ap()
    wpt_d = nc.dram_tensor("wpt", [GE, E], F32, kind="ExternalInput").ap()
    tri_d = nc.dram_tensor("tri", [KB, KB], F32, kind="ExternalInput").ap()
    ones_d = nc.dram_tensor("ones", [128, (T // KB) * GH], F32, kind="ExternalInput").ap()
    y_d = nc.dram_tensor("y", [T, E], F32, kind="ExternalOutput").ap()

    with tile.TileContext(nc) as tc:
        with tc.tile_pool(name="weights", bufs=1) as wpool, \
             tc.tile_pool(name="qk", bufs=1) as qkpool, \
             tc.tile_pool(name="vsb", bufs=1) as vpool, \
             tc.tile_pool(name="xin", bufs=o["x_bufs"]) as xpool, \
             tc.tile_pool(name="ptile", bufs=o["p_bufs"]) as ppool, \
             tc.tile_pool(name="osb", bufs=3) as opool, \
             tc.tile_pool(name="lbc", bufs=o.get("l_bufs", 3)) as lpool, \
             tc.tile_pool(name="onorm", bufs=o.get("on_bufs", 4)) as onpool, \
             tc.tile_pool(name="ystage", bufs=o.get("yst_bufs", 2)) as ypool:

            # --- weights / mask tiles (DMAs emitted inside phase 1 so x
            # transfers come first and matmuls start early) ---
            wq_sb = wpool.tile([128, KC, GE], F32R)
            wk_sb = wpool.tile([128, KC, GE], F32R)
            wv_sb = wpool.tile([128, KC, GE], F32R)
            wp_sb = wpool.tile([128, 2, E], F32R)
            tri_sb = wpool.tile([KB, KB], F32R)

            def load_weight_chunk(kc, which):
                for w_sb, w_d in which:
                    nc.sync.dma_start(
                        out=w_sb[:, kc, :],
                        in_=w_d.bitcast(F32R)[kc * 128:(kc + 1) * 128, :])

            qt_sb = qkpool.tile([128, 2, T], F32R)   # pair-stacked Q^T
            kt_sb = qkpool.tile([128, 2, T], F32R)   # pair-stacked K^T
            v_sb = vpool.tile([128, T // KB, GH * (D + 1)], F32R)  # [k-part, kblock, head-slot(64 V + 1 ones)]

            # ones columns of the augmented V (col D of each 65-wide head
            # slot); loaded after the first x chunks to keep startup clean
            v_ones = v_sb.rearrange("p b (h c) -> p (b h) c", c=D + 1)[:, :, D:D + 1]
            ones_sb = wpool.tile([128, (T // KB) * GH], F32R)
            nc.sync.dma_start(out=ones_sb[:], in_=ones_d.bitcast(F32R))
            nc.vector.tensor_copy(
                out=v_ones,
                in_=ones_sb.rearrange("p (n o) -> p n o", o=1),
            )

            # --- phase 1: projections ---
            proj_ctx = ExitStack()
            qk_ps = proj_ctx.enter_context(tc.tile_pool(name="qk_ps", bufs=o.get("qk_bufs", 2), space="PSUM"))
            v_ps = proj_ctx.enter_context(tc.tile_pool(name="v_ps", bufs=o.get("v_bufs", 2), space="PSUM"))
            for tch in range(NTCH):
                ts0 = tch * TCH
                xts = []
                split0 = o.get("x_split_first", False) and tch == 0
                for kc in range(KC):
                    xt = xpool.tile([128, TCH], F32R, tag="xt")
                    if split0:
                        # halved transfers so the first matmuls start sooner
                        for hf in range(2):
                            nc.sync.dma_start(
                                out=xt[:, hf * (TCH // 2):(hf + 1) * (TCH // 2)],
                                in_=xt_d.bitcast(F32R)[kc * 128:(kc + 1) * 128,
                                                       ts0 + hf * (TCH // 2):ts0 + (hf + 1) * (TCH // 2)])
                    else:
                        nc.sync.dma_start(out=xt[:], in_=xt_d.bitcast(F32R)[kc * 128:(kc + 1) * 128, ts0:ts0 + TCH])
                    xts.append(xt)
                    if tch == 0:
                        # q/k weights ride along with their x chunk; v weights
                        # (used later in the t-chunk) trail by 4 chunks
                        load_weight_chunk(kc, ((wq_sb, wqt_d), (wk_sb, wkt_d)))
                        if kc >= 4:
                            load_weight_chunk(kc - 4, ((wv_sb, wvt_d),))
                if tch == 0:
                    for kc in range(4, KC):
                        load_weight_chunk(kc, ((wv_sb, wvt_d),))
                    nc.sync.dma_start(out=tri_sb[:], in_=tri_d.bitcast(F32R))
                    nc.sync.dma_start(out=wp_sb[:], in_=wpt_d.bitcast(F32R).rearrange("(c p) n -> p c n", p=128))
                halves = ((0, TCH // 2), (TCH // 2, TCH)) if split0 else ((0, TCH),)
                for pair in range(2):
                    psl = slice(pair * 128, (pair + 1) * 128)
                    qp = qk_ps.tile([128, TCH], F32)
                    for h0, h1 in halves:
                        for kc in range(KC):
                            nc.tensor.matmul(qp[:, h0:h1], wq_sb[:, kc, psl], xts[kc][:, h0:h1],
                                             start=(kc == 0), stop=(kc == KC - 1))
                    nc.vector.tensor_copy(out=qt_sb[:, pair, ts0:ts0 + TCH], in_=qp[:])
                    kp = qk_ps.tile([128, TCH], F32)
                    for h0, h1 in halves:
                        for kc in range(KC):
                            nc.tensor.matmul(kp[:, h0:h1], wk_sb[:, kc, psl], xts[kc][:, h0:h1],
                                             start=(kc == 0), stop=(kc == KC - 1))
                    nc.vector.tensor_copy(out=kt_sb[:, pair, ts0:ts0 + TCH], in_=kp[:])
                for tsub in range(TCH // KB):
                    vp = v_ps.tile([128, GE], F32)
                    for kc in range(KC):
                        nc.tensor.matmul(vp[:], xts[kc][:, tsub * KB:(tsub + 1) * KB], wv_sb[:, kc, :],
                                         start=(kc == 0), stop=(kc == KC - 1))
                    tb = tch * (TCH // KB) + tsub
                    nc.vector.tensor_copy(
                        out=v_sb[:, tb, :].rearrange("p (h c) -> p h c", c=D + 1)[:, :, 0:D],
                        in_=vp.rearrange("p (h c) -> p h c", c=D),
                    )

            proj_ctx.close()

            # --- phase 2+3: attention per q-block, then its output projection ---
            attn_ctx = ExitStack()
            s_ps = attn_ctx.enter_context(tc.tile_pool(name="s_ps", bufs=o["s_bufs"], space="PSUM"))
            o_ps = attn_ctx.enter_context(tc.tile_pool(name="o_ps", bufs=o["o_bufs"], space="PSUM"))
            if o["y_in_s"]:
                y_ps, y_tag = s_ps, "s"
            elif o.get("y_in_o", False):
                y_ps, y_tag = o_ps, "o"
            else:
                y_ps = attn_ctx.enter_context(
                    tc.tile_pool(name="y_ps", bufs=o.get("y_bufs", 1), space="PSUM"))
                y_tag = "y"

            def slot(hb):
                return slice(hb * (D + 1), (hb + 1) * (D + 1))

            def tri_eng(pt, r):
                eng = nc.gpsimd if o.get("tri_gpsimd", False) else nc.vector
                eng.tensor_mul(pt[:, r:r + KB], pt[:, r:r + KB], tri_sb[:])

            def normalize(o_p, onorm, h, splits=1):
                # reciprocal of the l row (PSUM partition 64 -> SBUF partition
                # 0; DVE handles the base shift), gpsimd-broadcast across 64
                # partitions, then normalize straight out of PSUM into the
                # pair-stacked onorm tile (head B writes partitions 64:128).
                # splits>1 chops the chain along q so downstream Y matmuls
                # start on the first q-tile sooner (used for the last q-block).
                w = QB // splits
                for s in range(splits):
                    qs = slice(s * w, (s + 1) * w)
                    strip = lpool.tile([1, w], F32, tag="strip")
                    if o.get("recip_fast", True):
                        nc.vector.reciprocal_approx_fast(out=strip[:], in_=o_p[D:D + 1, qs])
                    else:
                        nc.vector.reciprocal(out=strip[:], in_=o_p[D:D + 1, qs])
                    lb = lpool.tile([D, w], F32, tag="lb")
                    nc.gpsimd.partition_broadcast(lb[:], strip[:])
                    nc.vector.tensor_mul(onorm[h * D:(h + 1) * D, qs], o_p[0:D, qs], lb[:])

            for qb in o.get("qb_order", list(range(NQB))):
                q0 = qb * QB
                nk = (q0 + QB) // KB          # kblocks 0..nk-1; last 4 are diagonal
                nfull = nk - 4
                onorms = []
                for pair in range(2):
                    onorm = onpool.tile([128, QB], F32R)
                    heads = [0, 1] if o["interleave"] else None
                    if o["interleave"]:
                        o_ps_t = [o_ps.tile([D + 1, QB], F32, tag="o", name="o_t") for _ in range(2)]
                        qr = [qt_sb[h * D:(h + 1) * D, pair, q0:q0 + QB] for h in range(2)]
                        # full k-blocks two at a time; S matmuls for the two
                        # heads adjacent (disjoint PE row groups -> HW overlap)
                        for j2 in range(0, nfull, 2):
                            sps = [s_ps.tile([128, 2 * QB], F32, tag="s", name="s_t") for _ in range(2)]
                            for jj in range(2):
                                j = j2 + jj
                                for h in range(2):
                                    nc.tensor.matmul(
                                        sps[h][:, jj * QB:(jj + 1) * QB],
                                        kt_sb[h * D:(h + 1) * D, pair, j * KB:(j + 1) * KB],
                                        qr[h], start=True, stop=True)
                            pts = []
                            for h in range(2):
                                pt = ppool.tile([128, 2 * QB], F32R, tag="p")
                                nc.scalar.activation(out=pt[:], in_=sps[h][:], func=EXP, scale=SCALE)
                                pts.append(pt)
                            for jj in range(2):
                                j = j2 + jj
                                for h in range(2):
                                    nc.tensor.matmul(
                                        o_ps_t[h][:], v_sb[:, j, slot(pair * 2 + h)],
                                        pts[h][:, jj * QB:(jj + 1) * QB],
                                        start=(j == 0), stop=False)
                        for j in range(nfull, nk):
                            r = (j - nfull) * KB
                            sps = [s_ps.tile([128, 2 * QB], F32, tag="s", name="s_t") for _ in range(2)]
                            for h in range(2):
                                nc.tensor.matmul(
                                    sps[h][:, 0:QB],
                                    kt_sb[h * D:(h + 1) * D, pair, j * KB:(j + 1) * KB],
                                    qr[h], start=True, stop=True)
                            for h in range(2):
                                pt = ppool.tile([128, 2 * QB], F32R, tag="p")
                                nc.scalar.activation(out=pt[:, r:QB], in_=sps[h][:, r:QB], func=EXP, scale=SCALE)
                                tri_eng(pt, r)
                                nc.tensor.matmul(
                                    o_ps_t[h][:, r:QB], v_sb[:, j, slot(pair * 2 + h)],
                                    pt[:, r:QB], start=(j == 0), stop=(j == nk - 1))
                        for h in range(2):
                            normalize(o_ps_t[h], onorm, h)
                    else:
                        for h in range(2):
                            hb = pair * 2 + h
                            bsl = slice(h * D, h * D + D)
                            o_p = o_ps.tile([D + 1, QB], F32)
                            qrhs = qt_sb[bsl, pair, q0:q0 + QB]
                            for j2 in range(0, nfull, 2):
                                sp = s_ps.tile([128, 2 * QB], F32, tag="s")
                                for jj in range(2):
                                    j = j2 + jj
                                    nc.tensor.matmul(sp[:, jj * QB:(jj + 1) * QB],
                                                     kt_sb[bsl, pair, j * KB:(j + 1) * KB],
                                                     qrhs, start=True, stop=True)
                                pt = ppool.tile([128, 2 * QB], F32R, tag="p")
                                nc.scalar.activation(out=pt[:], in_=sp[:], func=EXP, scale=SCALE)
                                for jj in range(2):
                                    j = j2 + jj
                                    nc.tensor.matmul(o_p[:], v_sb[:, j, slot(hb)],
                                                     pt[:, jj * QB:(jj + 1) * QB],
                                                     start=(j == 0), stop=False)
                            for j2 in range(nfull, nk, 2):
                                # two diagonal k-blocks share one 2-bank psum
                                # tile and a single exp over [r0 : QB+r1+KB]
                                # (the [QB : QB+r1) strip is unread garbage)
                                r0 = (j2 - nfull) * KB
                                r1 = r0 + KB
                                sp = s_ps.tile([128, 2 * QB], F32, tag="s")
                                for jj in range(2):
                                    j = j2 + jj
                                    nc.tensor.matmul(sp[:, jj * QB:(jj + 1) * QB],
                                                     kt_sb[bsl, pair, j * KB:(j + 1) * KB],
                                                     qrhs, start=True, stop=True)
                                pt = ppool.tile([128, 2 * QB], F32R, tag="p")
                                nc.scalar.activation(out=pt[:, r0:2 * QB], in_=sp[:, r0:2 * QB],
                                                     func=EXP, scale=SCALE)
                                for jj, r in ((0, r0), (1, r1)):
                                    j = j2 + jj
                                    base_c = jj * QB
                                    nc.vector.tensor_mul(pt[:, base_c + r:base_c + r + KB],
                                                         pt[:, base_c + r:base_c + r + KB], tri_sb[:])
                                    nc.tensor.matmul(o_p[:, r:QB], v_sb[:, j, slot(hb)],
                                                     pt[:, base_c + r:base_c + QB],
                                                     start=(j == 0), stop=(j == nk - 1))
                            normalize(o_p, onorm, h,
                                      splits=(o.get("norm_splits_last", 1) if qb == NQB - 1 else 1))
                    onorms.append(onorm)
                # output projection for this q-block
                for qt in range(QB // 128):
                    if o.get("y_split", False):
                        yt = ypool.tile([128, E], F32)
                        for nh in range(2):
                            if o.get("y_in_o", False):
                                yp = o_ps.tile([128, 512], F32, tag="o" if o["interleave"] else "o_p", name="yp")
                            elif o.get("y_last_in_s", False) and qb == NQB - 1:
                                yp = s_ps.tile([128, 512], F32, tag="s", name="yp")
                            else:
                                yp = y_ps.tile([128, 512], F32, tag=y_tag, name="yp")
                            for pair in range(2):
                                nc.tensor.matmul(yp[:],
                                                 onorms[pair][:, qt * 128:(qt + 1) * 128],
                                                 wp_sb[:, pair, nh * 512:(nh + 1) * 512],
                                                 start=(pair == 0), stop=(pair == 1))
                            if o.get("y_copy_act", False) and nh == 1:
                                nc.scalar.activation(out=yt[:, nh * 512:(nh + 1) * 512], in_=yp[:],
                                                     func=mybir.ActivationFunctionType.Copy)
                            else:
                                nc.vector.tensor_copy(out=yt[:, nh * 512:(nh + 1) * 512], in_=yp[:])
                        nc.sync.dma_start(out=y_d[q0 + qt * 128:q0 + (qt + 1) * 128, :], in_=yt[:])
                    else:
                        yp = y_ps.tile([128, E], F32, tag=y_tag)
                        for nh in range(2):
                            for pair in range(2):
                                nc.tensor.matmul(yp[:, nh * 512:(nh + 1) * 512],
                                                 onorms[pair][:, qt * 128:(qt + 1) * 128],
                                                 wp_sb[:, pair, nh * 512:(nh + 1) * 512],
                                                 start=(pair == 0), stop=(pair == 1))
                        yt = ypool.tile([128, E], F32)
                        nc.vector.tensor_copy(out=yt[:], in_=yp[:])
                        nc.sync.dma_start(out=y_d[q0 + qt * 128:q0 + (qt + 1) * 128, :], in_=yt[:])
            attn_ctx.close()

    nc.compile()
    return nc


_NC = {}


def _get_program(opts=None):
    key = tuple(sorted((opts or {}).items()))
    if key not in _NC:
        _NC[key] = build_program(opts)
    return _NC[key]


def _make_in_maps(x, Wq, Wk, Wv, Wp):
    x = np.asarray(x, dtype=np.float32)
    wqt = np.ascontiguousarray(np.asarray(Wq, np.float32).T)
    wkt = np.ascontiguousarray(np.asarray(Wk, np.float32).T)
    wvt = np.ascontiguousarray(np.asarray(Wv, np.float32).T)
    wpt = np.ascontiguousarray(np.asarray(Wp, np.float32).T)
    tri = (np.arange(KB)[:, None] <= np.arange(KB)[None, :]).astype(np.float32)
    ones = np.ones((128, (T // KB) * GH), np.float32)
    in_maps = []
    for c in range(N_CORES):
        b, hg = c // 4, c % 4
        in_maps.append({
            "xt": np.ascontiguousarray(x[b].T),
            "wqt": np.ascontiguousarray(wqt[:, hg * GE:(hg + 1) * GE]),
            "wkt": np.ascontiguousarray(wkt[:, hg * GE:(hg + 1) * GE]),
            "wvt": np.ascontiguousarray(wvt[:, hg * GE:(hg + 1) * GE]),
            "wpt": np.ascontiguousarray(wpt[hg * GE:(hg + 1) * GE, :]),
            "tri": tri,
            "ones": ones,
        })
    return in_maps


def run_cores(x, Wq, Wk, Wv, Wp, bp, **spmd_kwargs):
    """Run the 8-core program; returns (y_full, BassKernelResults)."""
    nc = _get_program()
    in_maps = _make_in_maps(x, Wq, Wk, Wv, Wp)
    res = run_bass_kernel_spmd(nc, in_maps, list(range(N_CORES)), **spmd_kwargs)
    parts = [res.results[c]["y"] for c in range(N_CORES)]
    y = np.empty((B, T, E), np.float32)
    for b in range(B):
        acc = parts[4 * b].astype(np.float32)
        for hg in range(1, 4):
            acc = acc + parts[4 * b + hg]
        y[b] = acc
    y += np.asarray(bp, np.float32)[None, None, :]
    return y, res


def kernel(x, Wq, Wk, Wv, Wp, bp):
    y, _ = run_cores(x, Wq, Wk, Wv, Wp, bp)
    return y


def bench(x, Wq, Wk, Wv, Wp, bp, iters=12):
    """Time repeated on-device executions of the compiled program.

    Returns (y_full, list_of_call_seconds). Builds the sharded jit once;
    inputs are device-resident; fresh donated zero outputs are staged
    outside the timed region each iteration.
    """
    import time

    import jax
    import numpy as np_
    from jax.experimental.shard_map import shard_map
    from jax.sharding import Mesh, NamedSharding, PartitionSpec

    from concourse import bass2jax, mybir as mb

    nc = _get_program()
    in_maps = _make_in_maps(x, Wq, Wk, Wv, Wp)
    n_cores = N_CORES
    bass2jax.install_neuronx_cc_hook()

    partition_name = nc.partition_id_tensor.name if nc.partition_id_tensor else None
    in_names, out_names, out_avals, zero_outs = [], [], [], []
    for alloc in nc.m.functions[0].allocations:
        if not isinstance(alloc, mb.MemoryLocationSet):
            continue
        name = alloc.memorylocations[0].name
        if alloc.kind == "ExternalInput":
            if name != partition_name:
                in_names.append(name)
        elif alloc.kind == "ExternalOutput":
            out_names.append(name)
            shape = tuple(alloc.tensor_shape)
            dtype = mb.dt.np(alloc.dtype)
            out_avals.append(jax.core.ShapedArray(shape, dtype))
            zero_outs.append(np_.zeros(shape, dtype))
    n_params = len(in_names)
    all_in_names = in_names + out_names
    if partition_name is not None:
        all_in_names = all_in_names + [partition_name]

    def _body(*args):
        operands = list(args)
        if partition_name is not None:
            operands.append(bass2jax.partition_id_tensor())
        outs = bass2jax._bass_exec_p.bind(
            *operands,
            out_avals=tuple(out_avals),
            in_names=tuple(all_in_names),
            out_names=tuple(out_names),
            lowering_input_output_aliases=(),
            sim_require_finite=True,
            sim_require_nnan=True,
            nc=nc,
        )
        return tuple(outs)

    devices = jax.devices()[:n_cores]
    mesh = Mesh(np_.asarray(devices), ("core",))
    donate = tuple(range(n_params, n_params + len(out_names)))
    sharded = jax.jit(
        shard_map(_body, mesh=mesh,
                  in_specs=(PartitionSpec("core"),) * (n_params + len(out_names)),
                  out_specs=(PartitionSpec("core"),) * len(out_names),
                  check_rep=False),
        donate_argnums=donate, keep_unused=True,
    )
    sh = NamedSharding(mesh, PartitionSpec("core"))
    concat_in = [
        jax.device_put(
            np_.concatenate([np_.asarray(in_maps[c][nm]) for c in range(n_cores)], axis=0), sh)
        for nm in in_names
    ]
    zeros_np = [np_.zeros((n_cores * z.shape[0], *z.shape[1:]), z.dtype) for z in zero_outs]

    times = []
    out_arrs = None
    for it in range(iters):
        dz = [jax.device_put(z, sh) for z in zeros_np]
        jax.block_until_ready(dz)
        t0 = time.perf_counter()
        out_arrs = sharded(*concat_in, *dz)
        jax.block_until_ready(out_arrs)
        times.append(time.perf_counter() - t0)

    parts = [
        np_.asarray(out_arrs[i]).reshape(n_cores, *out_avals[i].shape)
        for i, nm in enumerate(out_names)
    ]
    yi = out_names.index("y")
    y = np_.empty((B, T, E), np_.float32)
    for b in range(B):
        acc = parts[yi][4 * b].astype(np_.float32)
        for hg in range(1, 4):
            acc = acc + parts[yi][4 * b + hg]
        y[b] = acc
    y += np_.asarray(bp, np_.float32)[None, None, :]
    return y, times



# revision 30
# speedup vs baseline: 1.1738x; 1.1738x over previous
"""Multi-head causal attention (B=2, T=2048, E=1024, H=16, D=64) on 8 trn2 cores.

Sharding: core c -> batch b = c // 4, head-group hg = c % 4 (4 heads each).
Per-core: interleaved pipeline over t-chunks — project Q/K/V for chunk t, then
causal flash attention for q-block t (S^T[k,q] layout, softmax denominator via
a ones-augmented V matmul), then the row-parallel output projection emits a
partial [T, E] in bf16. Host sums the 4 partials per batch and adds bias.

Dtype split: stationary matmul operands (weights, K^T, V-aug, P-or-onorm) stay
float32r (self-loading, no Ldweights); moving operands (x, Q^T, P, Wp) are
bf16 (1 cyc/row at any width, half DMA).
"""
import sys
from contextlib import ExitStack

sys.path.insert(0, "/opt/trn_rl_repo")

import numpy as np
import ml_dtypes

import concourse.bass as bass
import concourse.tile as tile
from concourse import bacc, mybir
from concourse.bass_utils import run_bass_kernel_spmd

F32 = mybir.dt.float32
F32R = mybir.dt.float32r
BF = mybir.dt.bfloat16
BF_NP = ml_dtypes.bfloat16
EXP = mybir.ActivationFunctionType.Exp

B, T, E, H = 2, 2048, 1024, 16
D = E // H              # 64
N_CORES = 8
GH = 4                  # heads per core
GE = GH * D             # 256 per-core projection width
SCALE = float(D) ** -0.5

TCH = 512               # projection t-chunk == attention q-block
NTCH = T // TCH         # 4
KC = 8                  # contraction chunks of 128 over E
QB = 512
NQB = T // QB           # 4
KB = 128                # attention k-block


DEFAULT_OPTS = dict(
    pv_stream=False,    # True: P@V streams V (65-row matmuls per q-tile);
                        # False: P@V streams P (baseline o^T layout)
    proj_bf16=True,     # projection matmul group dtype (x, wq, wk, wv)
    attn_bf16=True,     # attention matmul group dtype (qt, kt, P, v, wp)
    defer_outproj=True,  # emit qb's out-projection after the next chunk's
                         # projections so proj matmuls cover normalize latency
    norm_splits_last=4,
    dma_spread=True,    # issue startup DMAs across SP/DVE/Pool queues
    act_preload=True,   # dummy exp so the act-table load happens at t=0
    s_bufs=2,
    o_bufs=2,
    aux_bufs=2,
    p_bufs=18,
    x_bufs=8,
    on_bufs=4,
    onn_bufs=4,
    l_bufs=8,
    y_bufs=3,
)


def build_program(opts=None):
    o = dict(DEFAULT_OPTS)
    if opts:
        o.update(opts)
    pv_stream = o["pv_stream"]
    # Legal dtype groups (HW verifier: matmul operands must share dtype when
    # fp32/f32r is involved):
    #   proj group (x, wq, wk, wv): bf16 (half DMA) or f32r
    #   attn group (qt, kt, P, v, onorm, wp, id): bf16 or f32r
    PROJ = BF if o["proj_bf16"] else F32R
    PROJ_D = BF if o["proj_bf16"] else F32
    ATTN = BF if o["attn_bf16"] else F32R
    ATTN_D = BF if o["attn_bf16"] else F32
    assert o["attn_bf16"] or not pv_stream, "pv_stream needs bf16 V"
    nc = bacc.Bacc("TRN2", target_bir_lowering=False, debug=False, num_devices=N_CORES)

    xt_d = nc.dram_tensor("xt", [E, T], PROJ_D, kind="ExternalInput").ap()
    wqt_d = nc.dram_tensor("wqt", [E, GE], PROJ_D, kind="ExternalInput").ap()
    wkt_d = nc.dram_tensor("wkt", [E, GE], PROJ_D, kind="ExternalInput").ap()
    wvt_d = nc.dram_tensor("wvt", [E, GE], PROJ_D, kind="ExternalInput").ap()
    wpt_d = nc.dram_tensor("wpt", [GE, E], ATTN_D, kind="ExternalInput").ap()
    tri_d = nc.dram_tensor("tri", [KB, KB], ATTN_D, kind="ExternalInput").ap()
    id_d = nc.dram_tensor("ident", [128, 128], ATTN_D, kind="ExternalInput").ap()
    y_d = nc.dram_tensor("y", [T, E], BF, kind="ExternalOutput").ap()

    def slot(hb):
        return slice(hb * (D + 1), (hb + 1) * (D + 1))

    with tile.TileContext(nc) as tc:
        ctx = ExitStack()
        wpool = ctx.enter_context(tc.tile_pool(name="weights", bufs=1))
        qkpool = ctx.enter_context(tc.tile_pool(name="qk", bufs=1))
        vpool = ctx.enter_context(tc.tile_pool(name="vsb", bufs=1))
        xpool = ctx.enter_context(tc.tile_pool(name="xin", bufs=o["x_bufs"]))
        ppool = ctx.enter_context(tc.tile_pool(name="ptile", bufs=o["p_bufs"]))
        onpool = ctx.enter_context(tc.tile_pool(name="onorm", bufs=o["on_bufs"]))
        onnpool = ctx.enter_context(tc.tile_pool(name="on", bufs=o["onn_bufs"]))
        lpool = ctx.enter_context(tc.tile_pool(name="lbc", bufs=o["l_bufs"]))
        ypool = ctx.enter_context(tc.tile_pool(name="ystage", bufs=o["y_bufs"]))
        s_ps = ctx.enter_context(tc.tile_pool(name="s_ps", bufs=o["s_bufs"], space="PSUM"))
        o_ps = ctx.enter_context(tc.tile_pool(name="o_ps", bufs=o["o_bufs"], space="PSUM"))
        aux_ps = ctx.enter_context(tc.tile_pool(name="aux_ps", bufs=o["aux_bufs"], space="PSUM"))

        wq_sb = wpool.tile([128, KC, GE], PROJ)
        wk_sb = wpool.tile([128, KC, GE], PROJ)
        wv_sb = wpool.tile([128, KC, GE], PROJ)
        wp_sb = wpool.tile([128, 2, E], ATTN)
        tri_sb = wpool.tile([KB, KB], ATTN)
        id_sb = wpool.tile([128, 128], ATTN)

        qt_sb = qkpool.tile([128, 2, T], ATTN)   # pair-stacked Q^T (moving)
        kt_sb = qkpool.tile([128, 2, T], ATTN)   # pair-stacked K^T (stationary)
        v_sb = vpool.tile([128, T // KB, GH * (D + 1)], ATTN)

        if o["act_preload"]:
            # Tiny exp at t=0 so the 1.3us act-table load happens during the
            # startup DMA window, not before the first real softmax.
            warm = wpool.tile([128, 1], F32)
            nc.gpsimd.memset(warm[:], 0.0)
            nc.scalar.activation(out=warm[:], in_=warm[:], func=EXP)

        # ones columns of the augmented V (col D of each 65-wide head slot)
        v_ones = v_sb.rearrange("p b (h c) -> p (b h) c", c=D + 1)[:, :, D:D + 1]
        nc.gpsimd.memset(v_ones, 1.0)

        P_DT = ATTN

        def mload(eng, out_ap, in_ap, r=(PROJ == F32R)):
            eng.dma_start(out=out_ap, in_=in_ap.bitcast(F32R) if r else in_ap)

        def normalize(o_p, onorm, h, splits=1):
            w = QB // splits
            for s in range(splits):
                qs = slice(s * w, (s + 1) * w)
                strip = lpool.tile([1, w], F32, tag="strip", name="strip")
                nc.vector.reciprocal(out=strip[:], in_=o_p[D:D + 1, qs])
                lb = lpool.tile([D, w], F32, tag="lb", name="lb")
                nc.gpsimd.partition_broadcast(lb[:], strip[:])
                nc.vector.tensor_mul(onorm[h * D:(h + 1) * D, qs], o_p[0:D, qs], lb[:])

        def emit_outproj(q0, onorms, stream_dma=False):
            yt = ypool.tile([128, QB // KB, E], BF, tag="y", name="yt")
            for qt in range(QB // KB):
                for nh in range(2):
                    yp = aux_ps.tile([128, 512], F32, tag="aux", name="yp")
                    for pair in range(2):
                        nc.tensor.matmul(yp[:],
                                         onorms[pair][:, qt * KB:(qt + 1) * KB],
                                         wp_sb[:, pair, nh * 512:(nh + 1) * 512],
                                         start=(pair == 0), stop=(pair == 1))
                    nc.vector.tensor_copy(out=yt[:, qt, nh * 512:(nh + 1) * 512], in_=yp[:])
                if stream_dma:
                    nc.sync.dma_start(out=y_d[q0 + qt * KB:q0 + (qt + 1) * KB, :],
                                      in_=yt[:, qt, :])
            if not stream_dma:
                nc.sync.dma_start(
                    out=y_d[q0:q0 + QB, :].rearrange("(a p) n -> p a n", p=128), in_=yt[:])

        prev_block = None  # (q0, onorms) awaiting out-projection
        for tch in range(NTCH):
            ts0 = tch * TCH
            # --- input DMAs (weights ride along with the first t-chunk) ---
            xb = xpool.tile([128, KC, TCH], PROJ, tag="xbig", name="xb", bufs=2)
            xr = xt_d.rearrange("(c p) t -> p c t", p=128)
            if tch == 0:
                # halved transfers interleaved with weight halves so the first
                # projection matmuls start as soon as possible
                wqr = wqt_d.rearrange("(c p) n -> p c n", p=128)
                wkr = wkt_d.rearrange("(c p) n -> p c n", p=128)
                wvr = wvt_d.rearrange("(c p) n -> p c n", p=128)
                mload(nc.sync, xb[:, 0:2, :], xr[:, 0:2, ts0:ts0 + TCH])
                mload(nc.sync, wq_sb[:, 0:2, :], wqr[:, 0:2, :])
                mload(nc.sync, xb[:, 2:4, :], xr[:, 2:4, ts0:ts0 + TCH])
                mload(nc.sync, wq_sb[:, 2:4, :], wqr[:, 2:4, :])
                mload(nc.sync, xb[:, 4:8, :], xr[:, 4:8, ts0:ts0 + TCH])
                mload(nc.sync, wq_sb[:, 4:8, :], wqr[:, 4:8, :])
                mload(nc.sync, wk_sb[:, 0:4, :], wkr[:, 0:4, :])
                mload(nc.sync, wk_sb[:, 4:8, :], wkr[:, 4:8, :])
                weng = nc.gpsimd if o["dma_spread"] else nc.sync
                mload(weng, wv_sb[:, 0:4, :], wvr[:, 0:4, :])
                mload(weng, wv_sb[:, 4:8, :], wvr[:, 4:8, :])
                aload = (ATTN == F32R)
                mload(nc.sync, tri_sb[:], tri_d, r=aload)
                mload(nc.sync, id_sb[:], id_d, r=aload)
                mload(nc.sync, wp_sb[:], wpt_d.rearrange("(c p) n -> p c n", p=128), r=aload)
            else:
                mload(nc.sync, xb[:], xr[:, :, ts0:ts0 + TCH])
            xts = [xb[:, kc, :] for kc in range(KC)]

            # --- Q/K/V projections for this t-chunk ---
            for pair in range(2):
                psl = slice(pair * 128, (pair + 1) * 128)
                qp = aux_ps.tile([128, TCH], F32, tag="aux", name="qp")
                for kc in range(KC):
                    nc.tensor.matmul(qp[:], wq_sb[:, kc, psl], xts[kc][:],
                                     start=(kc == 0), stop=(kc == KC - 1))
                nc.vector.tensor_copy(out=qt_sb[:, pair, ts0:ts0 + TCH], in_=qp[:])
                kp = aux_ps.tile([128, TCH], F32, tag="aux", name="kp")
                for kc in range(KC):
                    nc.tensor.matmul(kp[:], wk_sb[:, kc, psl], xts[kc][:],
                                     start=(kc == 0), stop=(kc == KC - 1))
                nc.vector.tensor_copy(out=kt_sb[:, pair, ts0:ts0 + TCH], in_=kp[:])
            for tsub in range(TCH // KB):
                vp = aux_ps.tile([128, GE], F32, tag="aux", name="vp")
                for kc in range(KC):
                    nc.tensor.matmul(vp[:], xts[kc][:, tsub * KB:(tsub + 1) * KB], wv_sb[:, kc, :],
                                     start=(kc == 0), stop=(kc == KC - 1))
                tb = tch * (TCH // KB) + tsub
                nc.vector.tensor_copy(
                    out=v_sb[:, tb, :].rearrange("p (h c) -> p h c", c=D + 1)[:, :, 0:D],
                    in_=vp.rearrange("p (h c) -> p h c", c=D),
                )

            # deferred out-projection of the previous q-block: the projection
            # matmuls above cover its normalize latency
            if o["defer_outproj"] and prev_block is not None:
                emit_outproj(*prev_block)
                prev_block = None

            # --- attention for q-block qb == tch ---
            qb = tch
            q0 = qb * QB
            nk = (q0 + QB) // KB
            nfull = nk - 4
            onorms = []
            for pair in range(2):
                onorm = onpool.tile([128, QB], ATTN, tag="onorm", name="onorm_t")
                ptiles = [[], []]
                for h in range(2):
                    bsl = slice(h * D, (h + 1) * D)
                    qr = qt_sb[bsl, pair, q0:q0 + QB]
                    for j2 in range(0, nfull, 2):
                        sp = s_ps.tile([128, 2, QB], F32, tag="s", name="sp")
                        for jj in range(2):
                            j = j2 + jj
                            nc.tensor.matmul(sp[:, jj, :],
                                             kt_sb[bsl, pair, j * KB:(j + 1) * KB],
                                             qr, start=True, stop=True)
                        pt = ppool.tile([128, 2, QB], P_DT, tag="p", name="pt")
                        nc.scalar.activation(out=pt.rearrange("p a b -> p (a b)"),
                                             in_=sp.rearrange("p a b -> p (a b)"),
                                             func=EXP, scale=SCALE)
                        ptiles[h].append(pt)
                    for j2 in range(nfull, nk, 2):
                        r0 = (j2 - nfull) * KB
                        r1 = r0 + KB
                        sp = s_ps.tile([128, 2, QB], F32, tag="s", name="sp")
                        nc.tensor.matmul(sp[:, 0, r0:QB],
                                         kt_sb[bsl, pair, j2 * KB:(j2 + 1) * KB],
                                         qr[:, r0:QB], start=True, stop=True)
                        nc.tensor.matmul(sp[:, 1, r1:QB],
                                         kt_sb[bsl, pair, (j2 + 1) * KB:(j2 + 2) * KB],
                                         qr[:, r1:QB], start=True, stop=True)
                        pt = ppool.tile([128, 2, QB], P_DT, tag="p", name="pt")
                        nc.scalar.activation(
                            out=pt.rearrange("p a b -> p (a b)")[:, r0:2 * QB],
                            in_=sp.rearrange("p a b -> p (a b)")[:, r0:2 * QB],
                            func=EXP, scale=SCALE)
                        nc.gpsimd.tensor_mul(pt[:, 0, r0:r0 + KB], pt[:, 0, r0:r0 + KB], tri_sb[:])
                        nc.gpsimd.tensor_mul(pt[:, 1, r1:r1 + KB], pt[:, 1, r1:r1 + KB], tri_sb[:])
                        ptiles[h].append(pt)

                if not pv_stream:
                    # P@V after BOTH heads' S: the second head's S matmuls buy
                    # the activation engine lead time over the first head's PV
                    for h in range(2):
                        o_p = o_ps.tile([D + 1, QB], F32, tag="o", name="o_t")
                        for j in range(nk):
                            j2, jj = divmod(j, 2)
                            r = max(0, (j - nfull) * KB)
                            nc.tensor.matmul(o_p[:, r:QB],
                                             v_sb[:, j, slot(pair * 2 + h)],
                                             ptiles[h][j2][:, jj, r:QB],
                                             start=(j == 0), stop=(j == nk - 1))
                        normalize(o_p, onorm, h,
                                  splits=(o["norm_splits_last"] if qb == NQB - 1 else 1))

                if pv_stream:
                    # P@V per 128-q-tile: stream the 65-wide augmented V; the
                    # P slabs are the (self-loading f32r) stationary side.
                    # Both heads of the pair accumulate into one PSUM bank.
                    for qt in range(QB // KB):
                        o_p = o_ps.tile([128, 2, 128], F32, tag="o", name="o_t")
                        jmax = nfull + qt
                        for h in range(2):
                            for j in range(jmax + 1):
                                j2, jj = divmod(j, 2)
                                nc.tensor.matmul(o_p[:, h, 0:D + 1],
                                                 ptiles[h][j2][:, jj, qt * KB:(qt + 1) * KB],
                                                 v_sb[:, j, slot(pair * 2 + h)],
                                                 start=(j == 0), stop=(j == jmax))
                        o_n = onnpool.tile([128, 2 * D], ATTN, tag="on", name="on_t")
                        for h in range(2):
                            linv = lpool.tile([128, 1], F32, tag="l", name="linv")
                            nc.vector.reciprocal(out=linv[:], in_=o_p[:, h, D:D + 1])
                            nc.vector.tensor_scalar_mul(o_n[:, h * D:(h + 1) * D],
                                                        o_p[:, h, 0:D], linv[:])
                        tp = aux_ps.tile([128, KB], ATTN, tag="aux", name="tp")
                        nc.tensor.transpose(tp[:], o_n[:], id_sb[:])
                        nc.vector.tensor_copy(out=onorm[:, qt * KB:(qt + 1) * KB], in_=tp[:])

                onorms.append(onorm)

            # --- output projection: either inline or deferred to the next
            # iteration (after its projections) ---
            if not o["defer_outproj"]:
                emit_outproj(q0, onorms)
            else:
                prev_block = (q0, onorms)

        if o["defer_outproj"] and prev_block is not None:
            emit_outproj(*prev_block, stream_dma=True)

        ctx.close()

    nc.compile()
    return nc


_NC = {}


def _get_program(opts=None):
    key = tuple(sorted((opts or {}).items()))
    if key not in _NC:
        _NC[key] = build_program(opts)
    return _NC[key]


def _make_in_maps(x, Wq, Wk, Wv, Wp, opts=None):
    o = dict(DEFAULT_OPTS)
    if opts:
        o.update(opts)
    pdt = BF_NP if o["proj_bf16"] else np.float32
    adt = BF_NP if o["attn_bf16"] else np.float32
    x = np.asarray(x, dtype=np.float32)
    wqt = np.asarray(Wq, np.float32).T
    wkt = np.asarray(Wk, np.float32).T
    wvt = np.asarray(Wv, np.float32).T
    wpt = np.asarray(Wp, np.float32).T
    tri = (np.arange(KB)[:, None] <= np.arange(KB)[None, :]).astype(adt)
    ident = np.eye(128, dtype=adt)
    in_maps = []
    for c in range(N_CORES):
        b, hg = c // 4, c % 4
        in_maps.append({
            "xt": np.ascontiguousarray(x[b].T).astype(pdt),
            "wqt": np.ascontiguousarray(wqt[:, hg * GE:(hg + 1) * GE]).astype(pdt),
            "wkt": np.ascontiguousarray(wkt[:, hg * GE:(hg + 1) * GE]).astype(pdt),
            "wvt": np.ascontiguousarray(wvt[:, hg * GE:(hg + 1) * GE]).astype(pdt),
            "wpt": np.ascontiguousarray(wpt[hg * GE:(hg + 1) * GE, :]).astype(adt),
            "tri": tri,
            "ident": ident,
        })
    return in_maps


def run_cores(x, Wq, Wk, Wv, Wp, bp, **spmd_kwargs):
    """Run the 8-core program; returns (y_full, BassKernelResults)."""
    nc = _get_program()
    in_maps = _make_in_maps(x, Wq, Wk, Wv, Wp)
    res = run_bass_kernel_spmd(nc, in_maps, list(range(N_CORES)), **spmd_kwargs)
    parts = [res.results[c]["y"] for c in range(N_CORES)]
    y = np.empty((B, T, E), np.float32)
    for b in range(B):
        acc = parts[4 * b].astype(np.float32)
        for hg in range(1, 4):
            acc = acc + parts[4 * b + hg].astype(np.float32)
        y[b] = acc
    y += np.asarray(bp, np.float32)[None, None, :]
    return y, res


def kernel(x, Wq, Wk, Wv, Wp, bp):
    y, _ = run_cores(x, Wq, Wk, Wv, Wp, bp)
    return y


def bench(x, Wq, Wk, Wv, Wp, bp, iters=12):
    """Time repeated on-device executions of the compiled program.

    Returns (y_full, list_of_call_seconds). Builds the sharded jit once;
    inputs are device-resident; fresh donated zero outputs are staged
    outside the timed region each iteration.
    """
    import time

    import jax
    import numpy as np_
    from jax.experimental.shard_map import shard_map
    from jax.sharding import Mesh, NamedSharding, PartitionSpec

    from concourse import bass2jax, mybir as mb

    nc = _get_program()
    in_maps = _make_in_maps(x, Wq, Wk, Wv, Wp)
    n_cores = N_CORES
    bass2jax.install_neuronx_cc_hook()

    partition_name = nc.partition_id_tensor.name if nc.partition_id_tensor else None
    in_names, out_names, out_avals, zero_outs = [], [], [], []
    for alloc in nc.m.functions[0].allocations:
        if not isinstance(alloc, mb.MemoryLocationSet):
            continue
        name = alloc.memorylocations[0].name
        if alloc.kind == "ExternalInput":
            if name != partition_name:
                in_names.append(name)
        elif alloc.kind == "ExternalOutput":
            out_names.append(name)
            shape = tuple(alloc.tensor_shape)
            dtype = mb.dt.np(alloc.dtype)
            out_avals.append(jax.core.ShapedArray(shape, dtype))
            zero_outs.append(np_.zeros(shape, dtype))
    n_params = len(in_names)
    all_in_names = in_names + out_names
    if partition_name is not None:
        all_in_names = all_in_names + [partition_name]

    def _body(*args):
        operands = list(args)
        if partition_name is not None:
            operands.append(bass2jax.partition_id_tensor())
        outs = bass2jax._bass_exec_p.bind(
            *operands,
            out_avals=tuple(out_avals),
            in_names=tuple(all_in_names),
            out_names=tuple(out_names),
            lowering_input_output_aliases=(),
            sim_require_finite=True,
            sim_require_nnan=True,
            nc=nc,
        )
        return tuple(outs)

    devices = jax.devices()[:n_cores]
    mesh = Mesh(np_.asarray(devices), ("core",))
    donate = tuple(range(n_params, n_params + len(out_names)))
    sharded = jax.jit(
        shard_map(_body, mesh=mesh,
                  in_specs=(PartitionSpec("core"),) * (n_params + len(out_names)),
                  out_specs=(PartitionSpec("core"),) * len(out_names),
                  check_rep=False),
        donate_argnums=donate, keep_unused=True,
    )
    sh = NamedSharding(mesh, PartitionSpec("core"))
    concat_in = [
        jax.device_put(
            np_.concatenate([np_.asarray(in_maps[c][nm]) for c in range(n_cores)], axis=0), sh)
        for nm in in_names
    ]
    zeros_np = [np_.zeros((n_cores * z.shape[0], *z.shape[1:]), z.dtype) for z in zero_outs]

    times = []
    out_arrs = None
    for it in range(iters):
        dz = [jax.device_put(z, sh) for z in zeros_np]
        jax.block_until_ready(dz)
        t0 = time.perf_counter()
        out_arrs = sharded(*concat_in, *dz)
        jax.block_until_ready(out_arrs)
        times.append(time.perf_counter() - t0)

    parts = [
        np_.asarray(out_arrs[i]).reshape(n_cores, *out_avals[i].shape)
        for i, nm in enumerate(out_names)
    ]
    yi = out_names.index("y")
    y = np_.empty((B, T, E), np_.float32)
    for b in range(B):
        acc = parts[yi][4 * b].astype(np_.float32)
        for hg in range(1, 4):
            acc = acc + parts[yi][4 * b + hg].astype(np_.float32)
        y[b] = acc
    y += np_.asarray(bp, np_.float32)[None, None, :]
    return y, times


# revision 37
# speedup vs baseline: 1.2578x; 1.0716x over previous
"""Multi-head causal attention (B=2, T=2048, E=1024, H=16, D=64) on 8 trn2 cores.

Sharding: core c -> batch b = c // 4, head-group hg = c % 4 (4 heads each).
Per-core: interleaved pipeline over t-chunks — project Q/K/V for chunk t, then
causal flash attention for q-block t (S^T[k,q] layout, softmax denominator via
a ones-augmented V matmul), then the row-parallel output projection emits a
partial [T, E] in bf16. Host sums the 4 partials per batch and adds bias.

Dtype split: stationary matmul operands (weights, K^T, V-aug, P-or-onorm) stay
float32r (self-loading, no Ldweights); moving operands (x, Q^T, P, Wp) are
bf16 (1 cyc/row at any width, half DMA).
"""
import sys
from contextlib import ExitStack

sys.path.insert(0, "/opt/trn_rl_repo")

import numpy as np
import ml_dtypes

import concourse.bass as bass
import concourse.tile as tile
from concourse import bacc, mybir
from concourse.bass_utils import run_bass_kernel_spmd

F32 = mybir.dt.float32
F32R = mybir.dt.float32r
BF = mybir.dt.bfloat16
F8 = mybir.dt.float8e4
BF_NP = ml_dtypes.bfloat16
EXP = mybir.ActivationFunctionType.Exp

B, T, E, H = 2, 2048, 1024, 16
D = E // H              # 64
N_CORES = 8
GH = 4                  # heads per core
GE = GH * D             # 256 per-core projection width
SCALE = float(D) ** -0.5

TCH = 512               # projection t-chunk == attention q-block
NTCH = T // TCH         # 4
KC = 8                  # contraction chunks of 128 over E
QB = 512
NQB = T // QB           # 4
KB = 128                # attention k-block


DEFAULT_OPTS = dict(
    pv_stream=False,    # True: P@V streams V (65-row matmuls per q-tile);
                        # False: P@V streams P (baseline o^T layout)
    proj_bf16=True,     # projection matmul group dtype (x, wq, wk, wv)
    attn_bf16=True,     # attention matmul group dtype (qt, kt, P, v, wp)
    defer_outproj=True,  # emit qb's out-projection after the next chunk's
                         # projections so proj matmuls cover normalize latency
    norm_splits_last=2,
    dma_spread=True,    # issue startup DMAs across SP/DVE/Pool queues
    act_preload=True,   # dummy exp so the act-table load happens at t=0
    s_bufs=2,
    o_bufs=2,
    aux_bufs=2,
    p_bufs=36,
    x_bufs=8,
    on_bufs=4,
    onn_bufs=4,
    l_bufs=8,
    y_bufs=3,
    v_before_k=True,
    xb_bufs=2,
    outproj_fill=True,
    exact_diag_exp=True,
    pair_interleave=True,
    s_fp8=True,
)


def build_program(opts=None):
    o = dict(DEFAULT_OPTS)
    if opts:
        o.update(opts)
    pv_stream = o["pv_stream"]
    # Legal dtype groups (HW verifier: matmul operands must share dtype when
    # fp32/f32r is involved):
    #   proj group (x, wq, wk, wv): bf16 (half DMA) or f32r
    #   attn group (qt, kt, P, v, onorm, wp, id): bf16 or f32r
    PROJ = BF if o["proj_bf16"] else F32R
    PROJ_D = BF if o["proj_bf16"] else F32
    ATTN = BF if o["attn_bf16"] else F32R
    ATTN_D = BF if o["attn_bf16"] else F32
    assert o["attn_bf16"] or not pv_stream, "pv_stream needs bf16 V"
    nc = bacc.Bacc("TRN2", target_bir_lowering=False, debug=False, num_devices=N_CORES)

    xt_d = nc.dram_tensor("xt", [E, T], PROJ_D, kind="ExternalInput").ap()
    wqt_d = nc.dram_tensor("wqt", [E, GE], PROJ_D, kind="ExternalInput").ap()
    wkt_d = nc.dram_tensor("wkt", [E, GE], PROJ_D, kind="ExternalInput").ap()
    wvt_d = nc.dram_tensor("wvt", [E, GE], PROJ_D, kind="ExternalInput").ap()
    wpt_d = nc.dram_tensor("wpt", [GE, E], ATTN_D, kind="ExternalInput").ap()
    tri_d = nc.dram_tensor("tri", [KB, KB], ATTN_D, kind="ExternalInput").ap()
    id_d = nc.dram_tensor("ident", [128, 128], ATTN_D, kind="ExternalInput").ap()
    y_d = nc.dram_tensor("y", [T, E], BF, kind="ExternalOutput").ap()

    def slot(hb):
        return slice(hb * (D + 1), (hb + 1) * (D + 1))

    with tile.TileContext(nc) as tc:
        ctx = ExitStack()
        wpool = ctx.enter_context(tc.tile_pool(name="weights", bufs=1))
        qkpool = ctx.enter_context(tc.tile_pool(name="qk", bufs=1))
        vpool = ctx.enter_context(tc.tile_pool(name="vsb", bufs=1))
        xpool = ctx.enter_context(tc.tile_pool(name="xin", bufs=o["x_bufs"]))
        ppool = ctx.enter_context(tc.tile_pool(name="ptile", bufs=o["p_bufs"]))
        onpool = ctx.enter_context(tc.tile_pool(name="onorm", bufs=o["on_bufs"]))
        onnpool = ctx.enter_context(tc.tile_pool(name="on", bufs=o["onn_bufs"]))
        lpool = ctx.enter_context(tc.tile_pool(name="lbc", bufs=o["l_bufs"]))
        ypool = ctx.enter_context(tc.tile_pool(name="ystage", bufs=o["y_bufs"]))
        s_ps = ctx.enter_context(tc.tile_pool(name="s_ps", bufs=o["s_bufs"], space="PSUM"))
        o_ps = ctx.enter_context(tc.tile_pool(name="o_ps", bufs=o["o_bufs"], space="PSUM"))
        aux_ps = ctx.enter_context(tc.tile_pool(name="aux_ps", bufs=o["aux_bufs"], space="PSUM"))

        wq_sb = wpool.tile([128, KC, GE], PROJ)
        wk_sb = wpool.tile([128, KC, GE], PROJ)
        wv_sb = wpool.tile([128, KC, GE], PROJ)
        wp_sb = wpool.tile([128, 2, E], ATTN)
        tri_sb = wpool.tile([KB, KB], ATTN)
        id_sb = wpool.tile([128, 128], ATTN)

        if o["s_fp8"]:
            # d-half-split layout: partitions hb*32:(hb+1)*32 hold head hb,
            # dim1 is the d-half -- the DoubleRow reduction pair
            qt_sb = qkpool.tile([128, 2, T], F8)
            kt_sb = qkpool.tile([128, 2, T], F8)
        else:
            qt_sb = qkpool.tile([128, 2, T], ATTN)   # pair-stacked Q^T (moving)
            kt_sb = qkpool.tile([128, 2, T], ATTN)   # pair-stacked K^T (stationary)
        v_sb = vpool.tile([128, T // KB, GH * (D + 1)], ATTN)

        if o["act_preload"]:
            # Tiny exp at t=0 so the 1.3us act-table load happens during the
            # startup DMA window, not before the first real softmax.
            warm = wpool.tile([128, 1], F32)
            nc.gpsimd.memset(warm[:], 0.0)
            nc.scalar.activation(out=warm[:], in_=warm[:], func=EXP)

        # ones columns of the augmented V (col D of each 65-wide head slot)
        v_ones = v_sb.rearrange("p b (h c) -> p (b h) c", c=D + 1)[:, :, D:D + 1]
        nc.gpsimd.memset(v_ones, 1.0)

        P_DT = ATTN

        def mload(eng, out_ap, in_ap, r=(PROJ == F32R)):
            eng.dma_start(out=out_ap, in_=in_ap.bitcast(F32R) if r else in_ap)

        def normalize(o_p, onorm, h, splits=1):
            w = QB // splits
            for s in range(splits):
                qs = slice(s * w, (s + 1) * w)
                strip = lpool.tile([1, w], F32, tag="strip", name="strip")
                nc.vector.reciprocal(out=strip[:], in_=o_p[D:D + 1, qs])
                lb = lpool.tile([D, w], F32, tag="lb", name="lb")
                nc.gpsimd.partition_broadcast(lb[:], strip[:])
                nc.vector.tensor_mul(onorm[h * D:(h + 1) * D, qs], o_p[0:D, qs], lb[:])

        def outproj_unit(yt, q0, onorms, qt, nh, stream_dma):
            yp = aux_ps.tile([128, 512], F32, tag="aux", name="yp")
            for pair in range(2):
                nc.tensor.matmul(yp[:],
                                 onorms[pair][:, qt * KB:(qt + 1) * KB],
                                 wp_sb[:, pair, nh * 512:(nh + 1) * 512],
                                 start=(pair == 0), stop=(pair == 1))
            nc.vector.tensor_copy(out=yt[:, qt, nh * 512:(nh + 1) * 512], in_=yp[:])
            if stream_dma and nh == 1:
                nc.sync.dma_start(out=y_d[q0 + qt * KB:q0 + (qt + 1) * KB, :],
                                  in_=yt[:, qt, :])

        def outproj_units(q0, onorms, stream_dma=False):
            yt = ypool.tile([128, QB // KB, E], BF, tag="y", name="yt")
            units = [(yt, q0, onorms, qt, nh, stream_dma)
                     for qt in range(QB // KB) for nh in range(2)]
            fin = []
            if not stream_dma:
                fin.append(lambda: nc.sync.dma_start(
                    out=y_d[q0:q0 + QB, :].rearrange("(a p) n -> p a n", p=128), in_=yt[:]))
            return units, fin

        def emit_outproj(q0, onorms, stream_dma=False):
            units, fin = outproj_units(q0, onorms, stream_dma)
            for u in units:
                outproj_unit(*u)
            for f in fin:
                f()

        prev_block = None  # (q0, onorms) awaiting out-projection
        for tch in range(NTCH):
            ts0 = tch * TCH
            # --- input DMAs (weights ride along with the first t-chunk) ---
            xb = xpool.tile([128, KC, TCH], PROJ, tag="xbig", name="xb", bufs=o["xb_bufs"])
            xr = xt_d.rearrange("(c p) t -> p c t", p=128)
            if tch == 0:
                # halved transfers interleaved with weight halves so the first
                # projection matmuls start as soon as possible
                wqr = wqt_d.rearrange("(c p) n -> p c n", p=128)
                wkr = wkt_d.rearrange("(c p) n -> p c n", p=128)
                wvr = wvt_d.rearrange("(c p) n -> p c n", p=128)
                mload(nc.sync, xb[:, 0:2, :], xr[:, 0:2, ts0:ts0 + TCH])
                mload(nc.sync, wq_sb[:, 0:2, :], wqr[:, 0:2, :])
                mload(nc.sync, xb[:, 2:4, :], xr[:, 2:4, ts0:ts0 + TCH])
                mload(nc.sync, wq_sb[:, 2:4, :], wqr[:, 2:4, :])
                mload(nc.sync, xb[:, 4:8, :], xr[:, 4:8, ts0:ts0 + TCH])
                mload(nc.sync, wq_sb[:, 4:8, :], wqr[:, 4:8, :])
                mload(nc.sync, wk_sb[:, 0:4, :], wkr[:, 0:4, :])
                mload(nc.sync, wk_sb[:, 4:8, :], wkr[:, 4:8, :])
                weng = nc.gpsimd if o["dma_spread"] else nc.sync
                mload(weng, wv_sb[:, 0:4, :], wvr[:, 0:4, :])
                mload(weng, wv_sb[:, 4:8, :], wvr[:, 4:8, :])
                aload = (ATTN == F32R)
                mload(nc.sync, tri_sb[:], tri_d, r=aload)
                mload(nc.sync, id_sb[:], id_d, r=aload)
                mload(nc.sync, wp_sb[:], wpt_d.rearrange("(c p) n -> p c n", p=128), r=aload)
            else:
                mload(nc.sync, xb[:], xr[:, :, ts0:ts0 + TCH])
            xts = [xb[:, kc, :] for kc in range(KC)]

            # --- Q/K/V projections for this t-chunk ---
            def proj_qk(which, w_sb, t_sb):
                for pair in range(2):
                    psl = slice(pair * 128, (pair + 1) * 128)
                    pp = aux_ps.tile([128, TCH], F32, tag="aux", name=which)
                    for kc in range(KC):
                        nc.tensor.matmul(pp[:], w_sb[:, kc, psl], xts[kc][:],
                                         start=(kc == 0), stop=(kc == KC - 1))
                    # pair-stacked (bf16 S) or d-half-split fp8 (DoubleRow S):
                    # the host reorders W columns so slot `pair` is the d-half
                    nc.vector.tensor_copy(out=t_sb[:, pair, ts0:ts0 + TCH], in_=pp[:])

            def proj_v():
                for tsub in range(TCH // KB):
                    vp = aux_ps.tile([128, GE], F32, tag="aux", name="vp")
                    for kc in range(KC):
                        nc.tensor.matmul(vp[:], xts[kc][:, tsub * KB:(tsub + 1) * KB],
                                         wv_sb[:, kc, :],
                                         start=(kc == 0), stop=(kc == KC - 1))
                    tb = tch * (TCH // KB) + tsub
                    nc.vector.tensor_copy(
                        out=v_sb[:, tb, :].rearrange("p (h c) -> p h c", c=D + 1)[:, :, 0:D],
                        in_=vp.rearrange("p (h c) -> p h c", c=D),
                    )

            proj_qk("qp", wq_sb, qt_sb)
            if o["v_before_k"]:
                proj_v()
                proj_qk("kp", wk_sb, kt_sb)
            else:
                proj_qk("kp", wk_sb, kt_sb)
                proj_v()

            # deferred out-projection of the previous q-block: either emitted
            # here (proj matmuls cover its normalize latency) or spread into
            # the attention stream as stall fillers
            fill_units, fill_fin = [], []
            if o["defer_outproj"] and prev_block is not None:
                if o["outproj_fill"]:
                    fill_units, fill_fin = outproj_units(*prev_block)
                    fill_units = list(fill_units)
                else:
                    emit_outproj(*prev_block)
                prev_block = None

            def fill(n=1):
                for _ in range(n):
                    if fill_units:
                        outproj_unit(*fill_units.pop(0))

            # --- attention for q-block qb == tch ---
            qb = tch
            q0 = qb * QB
            nk = (q0 + QB) // KB
            nfull = nk - 4
            onorms = []

            DR = mybir.MatmulPerfMode.DoubleRow

            def s_matmul(out_ap, pair, h, jsl, qsl):
                if o["s_fp8"]:
                    hb32 = (pair * 2 + h) * 32
                    hsl = slice(hb32, hb32 + 32)
                    nc.tensor.matmul(out_ap, kt_sb[hsl, :, jsl], qt_sb[hsl, :, qsl],
                                     start=True, stop=True, perf_mode=DR,
                                     tile_position=(hb32, 0))
                else:
                    bsl = slice(h * D, (h + 1) * D)
                    nc.tensor.matmul(out_ap, kt_sb[bsl, pair, jsl], qt_sb[bsl, pair, qsl],
                                     start=True, stop=True)

            def emit_s(pair, h, ptl):
                    for j2 in range(0, nfull, 2):
                        sp = s_ps.tile([128, 2, QB], F32, tag="s", name="sp")
                        for jj in range(2):
                            j = j2 + jj
                            s_matmul(sp[:, jj, :], pair, h,
                                     slice(j * KB, (j + 1) * KB), slice(q0, q0 + QB))
                        pt = ppool.tile([128, 2, QB], P_DT, tag="p", name="pt")
                        nc.scalar.activation(out=pt.rearrange("p a b -> p (a b)"),
                                             in_=sp.rearrange("p a b -> p (a b)"),
                                             func=EXP, scale=SCALE)
                        ptl.append(pt)
                    for j2 in range(nfull, nk, 2):
                        r0 = (j2 - nfull) * KB
                        r1 = r0 + KB
                        sp = s_ps.tile([128, 2, QB], F32, tag="s", name="sp")
                        s_matmul(sp[:, 0, r0:QB], pair, h,
                                 slice(j2 * KB, (j2 + 1) * KB), slice(q0 + r0, q0 + QB))
                        s_matmul(sp[:, 1, r1:QB], pair, h,
                                 slice((j2 + 1) * KB, (j2 + 2) * KB), slice(q0 + r1, q0 + QB))
                        pt = ppool.tile([128, 2, QB], P_DT, tag="p", name="pt")
                        if o["exact_diag_exp"]:
                            nc.scalar.activation(out=pt[:, 0, r0:QB], in_=sp[:, 0, r0:QB],
                                                 func=EXP, scale=SCALE)
                            nc.scalar.activation(out=pt[:, 1, r1:QB], in_=sp[:, 1, r1:QB],
                                                 func=EXP, scale=SCALE)
                        else:
                            nc.scalar.activation(
                                out=pt.rearrange("p a b -> p (a b)")[:, r0:2 * QB],
                                in_=sp.rearrange("p a b -> p (a b)")[:, r0:2 * QB],
                                func=EXP, scale=SCALE)
                        nc.gpsimd.tensor_mul(pt[:, 0, r0:r0 + KB], pt[:, 0, r0:r0 + KB], tri_sb[:])
                        nc.gpsimd.tensor_mul(pt[:, 1, r1:r1 + KB], pt[:, 1, r1:r1 + KB], tri_sb[:])
                        ptl.append(pt)

            def emit_pv(pair, h, ptl, onorm):
                    o_p = o_ps.tile([D + 1, QB], F32, tag="o", name="o_t")
                    for j in range(nfull):
                        j2, jj = divmod(j, 2)
                        nc.tensor.matmul(o_p[:],
                                         v_sb[:, j, slot(pair * 2 + h)],
                                         ptl[j2][:, jj, :],
                                         start=(j == 0), stop=False)
                    fill(1)
                    for j in range(nfull, nk):
                        j2, jj = divmod(j, 2)
                        r = (j - nfull) * KB
                        nc.tensor.matmul(o_p[:, r:QB],
                                         v_sb[:, j, slot(pair * 2 + h)],
                                         ptl[j2][:, jj, r:QB],
                                         start=(j == 0 if nfull == 0 else False),
                                         stop=(j == nk - 1))
                    normalize(o_p, onorm, h,
                              splits=(o["norm_splits_last"] if qb == NQB - 1 else 1))

            assert not pv_stream
            if o["pair_interleave"]:
                onorms = [onpool.tile([128, QB], ATTN, tag="onorm", name="onorm_t")
                          for _ in range(2)]
                ptls = {}
                for pair in range(2):
                    for h in range(2):
                        ptls[(pair, h)] = []
                        emit_s(pair, h, ptls[(pair, h)])
                    fill(1)
                for pair in range(2):
                    for h in range(2):
                        emit_pv(pair, h, ptls[(pair, h)], onorms[pair])
                    fill(1)
            else:
                for pair in range(2):
                    onorm = onpool.tile([128, QB], ATTN, tag="onorm", name="onorm_t")
                    ptls = [[], []]
                    for h in range(2):
                        emit_s(pair, h, ptls[h])
                    fill(2)
                    for h in range(2):
                        emit_pv(pair, h, ptls[h], onorm)
                    onorms.append(onorm)


            for u in fill_units:
                outproj_unit(*u)
            for f in fill_fin:
                f()

            # --- output projection: either inline or deferred to the next
            # iteration (after its projections) ---
            if not o["defer_outproj"]:
                emit_outproj(q0, onorms)
            else:
                prev_block = (q0, onorms)

        if o["defer_outproj"] and prev_block is not None:
            emit_outproj(*prev_block, stream_dma=True)

        ctx.close()

    nc.compile()
    return nc


_NC = {}


def _get_program(opts=None):
    key = tuple(sorted((opts or {}).items()))
    if key not in _NC:
        _NC[key] = build_program(opts)
    return _NC[key]


def _make_in_maps(x, Wq, Wk, Wv, Wp, opts=None):
    o = dict(DEFAULT_OPTS)
    if opts:
        o.update(opts)
    pdt = BF_NP if o["proj_bf16"] else np.float32
    adt = BF_NP if o["attn_bf16"] else np.float32
    x = np.asarray(x, dtype=np.float32)
    wqt = np.asarray(Wq, np.float32).T
    wkt = np.asarray(Wk, np.float32).T
    if o["s_fp8"]:
        # reorder per-core GE columns to the d-half-split layout:
        # half-major, then head, then d-within-half
        perm = np.array([hb * 64 + half * 32 + d
                         for half in range(2) for hb in range(4) for d in range(32)])
    else:
        perm = np.arange(GE)
    wvt = np.asarray(Wv, np.float32).T
    wpt = np.asarray(Wp, np.float32).T
    tri = (np.arange(KB)[:, None] <= np.arange(KB)[None, :]).astype(adt)
    ident = np.eye(128, dtype=adt)
    in_maps = []
    for c in range(N_CORES):
        b, hg = c // 4, c % 4
        in_maps.append({
            "xt": np.ascontiguousarray(x[b].T).astype(pdt),
            "wqt": np.ascontiguousarray(wqt[:, hg * GE:(hg + 1) * GE][:, perm]).astype(pdt),
            "wkt": np.ascontiguousarray(wkt[:, hg * GE:(hg + 1) * GE][:, perm]).astype(pdt),
            "wvt": np.ascontiguousarray(wvt[:, hg * GE:(hg + 1) * GE]).astype(pdt),
            "wpt": np.ascontiguousarray(wpt[hg * GE:(hg + 1) * GE, :]).astype(adt),
            "tri": tri,
            "ident": ident,
        })
    return in_maps


def run_cores(x, Wq, Wk, Wv, Wp, bp, **spmd_kwargs):
    """Run the 8-core program; returns (y_full, BassKernelResults)."""
    nc = _get_program()
    in_maps = _make_in_maps(x, Wq, Wk, Wv, Wp)
    res = run_bass_kernel_spmd(nc, in_maps, list(range(N_CORES)), **spmd_kwargs)
    parts = [res.results[c]["y"] for c in range(N_CORES)]
    y = np.empty((B, T, E), np.float32)
    for b in range(B):
        acc = parts[4 * b].astype(np.float32)
        for hg in range(1, 4):
            acc = acc + parts[4 * b + hg].astype(np.float32)
        y[b] = acc
    y += np.asarray(bp, np.float32)[None, None, :]
    return y, res


def kernel(x, Wq, Wk, Wv, Wp, bp):
    y, _ = run_cores(x, Wq, Wk, Wv, Wp, bp)
    return y


def bench(x, Wq, Wk, Wv, Wp, bp, iters=12):
    """Time repeated on-device executions of the compiled program.

    Returns (y_full, list_of_call_seconds). Builds the sharded jit once;
    inputs are device-resident; fresh donated zero outputs are staged
    outside the timed region each iteration.
    """
    import time

    import jax
    import numpy as np_
    from jax.experimental.shard_map import shard_map
    from jax.sharding import Mesh, NamedSharding, PartitionSpec

    from concourse import bass2jax, mybir as mb

    nc = _get_program()
    in_maps = _make_in_maps(x, Wq, Wk, Wv, Wp)
    n_cores = N_CORES
    bass2jax.install_neuronx_cc_hook()

    partition_name = nc.partition_id_tensor.name if nc.partition_id_tensor else None
    in_names, out_names, out_avals, zero_outs = [], [], [], []
    for alloc in nc.m.functions[0].allocations:
        if not isinstance(alloc, mb.MemoryLocationSet):
            continue
        name = alloc.memorylocations[0].name
        if alloc.kind == "ExternalInput":
            if name != partition_name:
                in_names.append(name)
        elif alloc.kind == "ExternalOutput":
            out_names.append(name)
            shape = tuple(alloc.tensor_shape)
            dtype = mb.dt.np(alloc.dtype)
            out_avals.append(jax.core.ShapedArray(shape, dtype))
            zero_outs.append(np_.zeros(shape, dtype))
    n_params = len(in_names)
    all_in_names = in_names + out_names
    if partition_name is not None:
        all_in_names = all_in_names + [partition_name]

    def _body(*args):
        operands = list(args)
        if partition_name is not None:
            operands.append(bass2jax.partition_id_tensor())
        outs = bass2jax._bass_exec_p.bind(
            *operands,
            out_avals=tuple(out_avals),
            in_names=tuple(all_in_names),
            out_names=tuple(out_names),
            lowering_input_output_aliases=(),
            sim_require_finite=True,
            sim_require_nnan=True,
            nc=nc,
        )
        return tuple(outs)

    devices = jax.devices()[:n_cores]
    mesh = Mesh(np_.asarray(devices), ("core",))
    donate = tuple(range(n_params, n_params + len(out_names)))
    sharded = jax.jit(
        shard_map(_body, mesh=mesh,
                  in_specs=(PartitionSpec("core"),) * (n_params + len(out_names)),
                  out_specs=(PartitionSpec("core"),) * len(out_names),
                  check_rep=False),
        donate_argnums=donate, keep_unused=True,
    )
    sh = NamedSharding(mesh, PartitionSpec("core"))
    concat_in = [
        jax.device_put(
            np_.concatenate([np_.asarray(in_maps[c][nm]) for c in range(n_cores)], axis=0), sh)
        for nm in in_names
    ]
    zeros_np = [np_.zeros((n_cores * z.shape[0], *z.shape[1:]), z.dtype) for z in zero_outs]

    times = []
    out_arrs = None
    for it in range(iters):
        dz = [jax.device_put(z, sh) for z in zeros_np]
        jax.block_until_ready(dz)
        t0 = time.perf_counter()
        out_arrs = sharded(*concat_in, *dz)
        jax.block_until_ready(out_arrs)
        times.append(time.perf_counter() - t0)

    parts = [
        np_.asarray(out_arrs[i]).reshape(n_cores, *out_avals[i].shape)
        for i, nm in enumerate(out_names)
    ]
    yi = out_names.index("y")
    y = np_.empty((B, T, E), np_.float32)
    for b in range(B):
        acc = parts[yi][4 * b].astype(np_.float32)
        for hg in range(1, 4):
            acc = acc + parts[yi][4 * b + hg].astype(np_.float32)
        y[b] = acc
    y += np_.asarray(bp, np_.float32)[None, None, :]
    return y, times


# revision 38
# speedup vs baseline: 1.2768x; 1.0151x over previous
"""Multi-head causal attention (B=2, T=2048, E=1024, H=16, D=64) on 8 trn2 cores.

Sharding: core c -> batch b = c // 4, head-group hg = c % 4 (4 heads each).
Per-core: interleaved pipeline over t-chunks — project Q/K/V for chunk t (bf16,
batched DMAs), then causal flash attention for q-block t (S^T[k,q] layout;
Q^T/K^T quantized to fp8e4 in a d-half-split layout so S runs as DoubleRow
matmuls at 0.5 cyc/row; softmax denominator via a ones-augmented V matmul;
P@V and the row-parallel output projection in bf16). Each q-block's output
projection is deferred into the next iteration and interleaved into the
attention stream as stall fillers; the partial [T, E] leaves in bf16 and the
host sums the 4 partials per batch and adds the bias.
"""
import sys
from contextlib import ExitStack

sys.path.insert(0, "/opt/trn_rl_repo")

import numpy as np
import ml_dtypes

import concourse.bass as bass
import concourse.tile as tile
from concourse import bacc, mybir
from concourse.bass_utils import run_bass_kernel_spmd

F32 = mybir.dt.float32
F32R = mybir.dt.float32r
BF = mybir.dt.bfloat16
F8 = mybir.dt.float8e4
BF_NP = ml_dtypes.bfloat16
EXP = mybir.ActivationFunctionType.Exp

B, T, E, H = 2, 2048, 1024, 16
D = E // H              # 64
N_CORES = 8
GH = 4                  # heads per core
GE = GH * D             # 256 per-core projection width
SCALE = float(D) ** -0.5

TCH = 512               # projection t-chunk == attention q-block
NTCH = T // TCH         # 4
KC = 8                  # contraction chunks of 128 over E
QB = 512
NQB = T // QB           # 4
KB = 128                # attention k-block


DEFAULT_OPTS = dict(
    pv_stream=False,    # True: P@V streams V (65-row matmuls per q-tile);
                        # False: P@V streams P (baseline o^T layout)
    proj_bf16=True,     # projection matmul group dtype (x, wq, wk, wv)
    attn_bf16=True,     # attention matmul group dtype (qt, kt, P, v, wp)
    defer_outproj=True,  # emit qb's out-projection after the next chunk's
                         # projections so proj matmuls cover normalize latency
    norm_splits_last=2,
    dma_spread=True,    # issue startup DMAs across SP/DVE/Pool queues
    act_preload=True,   # dummy exp so the act-table load happens at t=0
    s_bufs=2,
    o_bufs=2,
    aux_bufs=2,
    p_bufs=40,
    x_bufs=8,
    on_bufs=4,
    onn_bufs=4,
    l_bufs=8,
    y_bufs=3,
    v_before_k=False,
    xb_bufs=2,
    outproj_fill=True,
    exact_diag_exp=False,
    pair_interleave=True,
    s_fp8=True,
)


def build_program(opts=None):
    o = dict(DEFAULT_OPTS)
    if opts:
        o.update(opts)
    pv_stream = o["pv_stream"]
    # Legal dtype groups (HW verifier: matmul operands must share dtype when
    # fp32/f32r is involved):
    #   proj group (x, wq, wk, wv): bf16 (half DMA) or f32r
    #   attn group (qt, kt, P, v, onorm, wp, id): bf16 or f32r
    PROJ = BF if o["proj_bf16"] else F32R
    PROJ_D = BF if o["proj_bf16"] else F32
    ATTN = BF if o["attn_bf16"] else F32R
    ATTN_D = BF if o["attn_bf16"] else F32
    assert o["attn_bf16"] or not pv_stream, "pv_stream needs bf16 V"
    nc = bacc.Bacc("TRN2", target_bir_lowering=False, debug=False, num_devices=N_CORES)

    xt_d = nc.dram_tensor("xt", [E, T], PROJ_D, kind="ExternalInput").ap()
    wqt_d = nc.dram_tensor("wqt", [E, GE], PROJ_D, kind="ExternalInput").ap()
    wkt_d = nc.dram_tensor("wkt", [E, GE], PROJ_D, kind="ExternalInput").ap()
    wvt_d = nc.dram_tensor("wvt", [E, GE], PROJ_D, kind="ExternalInput").ap()
    wpt_d = nc.dram_tensor("wpt", [GE, E], ATTN_D, kind="ExternalInput").ap()
    tri_d = nc.dram_tensor("tri", [KB, KB], ATTN_D, kind="ExternalInput").ap()
    id_d = nc.dram_tensor("ident", [128, 128], ATTN_D, kind="ExternalInput").ap()
    y_d = nc.dram_tensor("y", [T, E], BF, kind="ExternalOutput").ap()

    def slot(hb):
        return slice(hb * (D + 1), (hb + 1) * (D + 1))

    with tile.TileContext(nc) as tc:
        ctx = ExitStack()
        wpool = ctx.enter_context(tc.tile_pool(name="weights", bufs=1))
        qkpool = ctx.enter_context(tc.tile_pool(name="qk", bufs=1))
        vpool = ctx.enter_context(tc.tile_pool(name="vsb", bufs=1))
        xpool = ctx.enter_context(tc.tile_pool(name="xin", bufs=o["x_bufs"]))
        ppool = ctx.enter_context(tc.tile_pool(name="ptile", bufs=o["p_bufs"]))
        onpool = ctx.enter_context(tc.tile_pool(name="onorm", bufs=o["on_bufs"]))
        onnpool = ctx.enter_context(tc.tile_pool(name="on", bufs=o["onn_bufs"]))
        lpool = ctx.enter_context(tc.tile_pool(name="lbc", bufs=o["l_bufs"]))
        ypool = ctx.enter_context(tc.tile_pool(name="ystage", bufs=o["y_bufs"]))
        s_ps = ctx.enter_context(tc.tile_pool(name="s_ps", bufs=o["s_bufs"], space="PSUM"))
        o_ps = ctx.enter_context(tc.tile_pool(name="o_ps", bufs=o["o_bufs"], space="PSUM"))
        aux_ps = ctx.enter_context(tc.tile_pool(name="aux_ps", bufs=o["aux_bufs"], space="PSUM"))

        wq_sb = wpool.tile([128, KC, GE], PROJ)
        wk_sb = wpool.tile([128, KC, GE], PROJ)
        wv_sb = wpool.tile([128, KC, GE], PROJ)
        wp_sb = wpool.tile([128, 2, E], ATTN)
        tri_sb = wpool.tile([KB, KB], ATTN)
        id_sb = wpool.tile([128, 128], ATTN)

        if o["s_fp8"]:
            # d-half-split layout: partitions hb*32:(hb+1)*32 hold head hb,
            # dim1 is the d-half -- the DoubleRow reduction pair
            qt_sb = qkpool.tile([128, 2, T], F8)
            kt_sb = qkpool.tile([128, 2, T], F8)
        else:
            qt_sb = qkpool.tile([128, 2, T], ATTN)   # pair-stacked Q^T (moving)
            kt_sb = qkpool.tile([128, 2, T], ATTN)   # pair-stacked K^T (stationary)
        v_sb = vpool.tile([128, T // KB, GH * (D + 1)], ATTN)

        if o["act_preload"]:
            # Tiny exp at t=0 so the 1.3us act-table load happens during the
            # startup DMA window, not before the first real softmax.
            warm = wpool.tile([128, 1], F32)
            nc.gpsimd.memset(warm[:], 0.0)
            nc.scalar.activation(out=warm[:], in_=warm[:], func=EXP)

        # ones columns of the augmented V (col D of each 65-wide head slot)
        v_ones = v_sb.rearrange("p b (h c) -> p (b h) c", c=D + 1)[:, :, D:D + 1]
        nc.gpsimd.memset(v_ones, 1.0)

        P_DT = ATTN

        def mload(eng, out_ap, in_ap, r=(PROJ == F32R)):
            eng.dma_start(out=out_ap, in_=in_ap.bitcast(F32R) if r else in_ap)

        def normalize(o_p, onorm, h, splits=1):
            w = QB // splits
            for s in range(splits):
                qs = slice(s * w, (s + 1) * w)
                strip = lpool.tile([1, w], F32, tag="strip", name="strip")
                nc.vector.reciprocal(out=strip[:], in_=o_p[D:D + 1, qs])
                lb = lpool.tile([D, w], F32, tag="lb", name="lb")
                nc.gpsimd.partition_broadcast(lb[:], strip[:])
                nc.vector.tensor_mul(onorm[h * D:(h + 1) * D, qs], o_p[0:D, qs], lb[:])

        def outproj_unit(yt, q0, onorms, qt, nh, stream_dma):
            yp = aux_ps.tile([128, 512], F32, tag="aux", name="yp")
            for pair in range(2):
                nc.tensor.matmul(yp[:],
                                 onorms[pair][:, qt * KB:(qt + 1) * KB],
                                 wp_sb[:, pair, nh * 512:(nh + 1) * 512],
                                 start=(pair == 0), stop=(pair == 1))
            nc.vector.tensor_copy(out=yt[:, qt, nh * 512:(nh + 1) * 512], in_=yp[:])
            if stream_dma and nh == 1:
                nc.sync.dma_start(out=y_d[q0 + qt * KB:q0 + (qt + 1) * KB, :],
                                  in_=yt[:, qt, :])

        def outproj_units(q0, onorms, stream_dma=False):
            yt = ypool.tile([128, QB // KB, E], BF, tag="y", name="yt")
            units = [(yt, q0, onorms, qt, nh, stream_dma)
                     for qt in range(QB // KB) for nh in range(2)]
            fin = []
            if not stream_dma:
                fin.append(lambda: nc.sync.dma_start(
                    out=y_d[q0:q0 + QB, :].rearrange("(a p) n -> p a n", p=128), in_=yt[:]))
            return units, fin

        def emit_outproj(q0, onorms, stream_dma=False):
            units, fin = outproj_units(q0, onorms, stream_dma)
            for u in units:
                outproj_unit(*u)
            for f in fin:
                f()

        prev_block = None  # (q0, onorms) awaiting out-projection
        for tch in range(NTCH):
            ts0 = tch * TCH
            # --- input DMAs (weights ride along with the first t-chunk) ---
            xb = xpool.tile([128, KC, TCH], PROJ, tag="xbig", name="xb", bufs=o["xb_bufs"])
            xr = xt_d.rearrange("(c p) t -> p c t", p=128)
            if tch == 0:
                # halved transfers interleaved with weight halves so the first
                # projection matmuls start as soon as possible
                wqr = wqt_d.rearrange("(c p) n -> p c n", p=128)
                wkr = wkt_d.rearrange("(c p) n -> p c n", p=128)
                wvr = wvt_d.rearrange("(c p) n -> p c n", p=128)
                mload(nc.sync, xb[:, 0:2, :], xr[:, 0:2, ts0:ts0 + TCH])
                mload(nc.sync, wq_sb[:, 0:2, :], wqr[:, 0:2, :])
                mload(nc.sync, xb[:, 2:4, :], xr[:, 2:4, ts0:ts0 + TCH])
                mload(nc.sync, wq_sb[:, 2:4, :], wqr[:, 2:4, :])
                mload(nc.sync, xb[:, 4:8, :], xr[:, 4:8, ts0:ts0 + TCH])
                mload(nc.sync, wq_sb[:, 4:8, :], wqr[:, 4:8, :])
                mload(nc.sync, wk_sb[:, 0:4, :], wkr[:, 0:4, :])
                mload(nc.sync, wk_sb[:, 4:8, :], wkr[:, 4:8, :])
                weng = nc.gpsimd if o["dma_spread"] else nc.sync
                mload(weng, wv_sb[:, 0:4, :], wvr[:, 0:4, :])
                mload(weng, wv_sb[:, 4:8, :], wvr[:, 4:8, :])
                aload = (ATTN == F32R)
                mload(nc.sync, tri_sb[:], tri_d, r=aload)
                mload(nc.sync, id_sb[:], id_d, r=aload)
                mload(nc.sync, wp_sb[:], wpt_d.rearrange("(c p) n -> p c n", p=128), r=aload)
            else:
                mload(nc.sync, xb[:], xr[:, :, ts0:ts0 + TCH])
            xts = [xb[:, kc, :] for kc in range(KC)]

            # --- Q/K/V projections for this t-chunk ---
            def proj_qk(which, w_sb, t_sb):
                for pair in range(2):
                    psl = slice(pair * 128, (pair + 1) * 128)
                    pp = aux_ps.tile([128, TCH], F32, tag="aux", name=which)
                    for kc in range(KC):
                        nc.tensor.matmul(pp[:], w_sb[:, kc, psl], xts[kc][:],
                                         start=(kc == 0), stop=(kc == KC - 1))
                    # pair-stacked (bf16 S) or d-half-split fp8 (DoubleRow S):
                    # the host reorders W columns so slot `pair` is the d-half
                    nc.vector.tensor_copy(out=t_sb[:, pair, ts0:ts0 + TCH], in_=pp[:])

            def proj_v():
                for tsub in range(TCH // KB):
                    vp = aux_ps.tile([128, GE], F32, tag="aux", name="vp")
                    for kc in range(KC):
                        nc.tensor.matmul(vp[:], xts[kc][:, tsub * KB:(tsub + 1) * KB],
                                         wv_sb[:, kc, :],
                                         start=(kc == 0), stop=(kc == KC - 1))
                    tb = tch * (TCH // KB) + tsub
                    nc.vector.tensor_copy(
                        out=v_sb[:, tb, :].rearrange("p (h c) -> p h c", c=D + 1)[:, :, 0:D],
                        in_=vp.rearrange("p (h c) -> p h c", c=D),
                    )

            proj_qk("qp", wq_sb, qt_sb)
            if o["v_before_k"]:
                proj_v()
                proj_qk("kp", wk_sb, kt_sb)
            else:
                proj_qk("kp", wk_sb, kt_sb)
                proj_v()

            # deferred out-projection of the previous q-block: either emitted
            # here (proj matmuls cover its normalize latency) or spread into
            # the attention stream as stall fillers
            fill_units, fill_fin = [], []
            if o["defer_outproj"] and prev_block is not None:
                if o["outproj_fill"]:
                    fill_units, fill_fin = outproj_units(*prev_block)
                    fill_units = list(fill_units)
                else:
                    emit_outproj(*prev_block)
                prev_block = None

            def fill(n=1):
                for _ in range(n):
                    if fill_units:
                        outproj_unit(*fill_units.pop(0))

            # --- attention for q-block qb == tch ---
            qb = tch
            q0 = qb * QB
            nk = (q0 + QB) // KB
            nfull = nk - 4
            onorms = []

            DR = mybir.MatmulPerfMode.DoubleRow

            def s_matmul(out_ap, pair, h, jsl, qsl):
                if o["s_fp8"]:
                    hb32 = (pair * 2 + h) * 32
                    hsl = slice(hb32, hb32 + 32)
                    nc.tensor.matmul(out_ap, kt_sb[hsl, :, jsl], qt_sb[hsl, :, qsl],
                                     start=True, stop=True, perf_mode=DR,
                                     tile_position=(hb32, 0))
                else:
                    bsl = slice(h * D, (h + 1) * D)
                    nc.tensor.matmul(out_ap, kt_sb[bsl, pair, jsl], qt_sb[bsl, pair, qsl],
                                     start=True, stop=True)

            def emit_s(pair, h, ptl):
                    for j2 in range(0, nfull, 2):
                        sp = s_ps.tile([128, 2, QB], F32, tag="s", name="sp")
                        for jj in range(2):
                            j = j2 + jj
                            s_matmul(sp[:, jj, :], pair, h,
                                     slice(j * KB, (j + 1) * KB), slice(q0, q0 + QB))
                        pt = ppool.tile([128, 2, QB], P_DT, tag="p", name="pt")
                        nc.scalar.activation(out=pt.rearrange("p a b -> p (a b)"),
                                             in_=sp.rearrange("p a b -> p (a b)"),
                                             func=EXP, scale=SCALE)
                        ptl.append(pt)
                    for j2 in range(nfull, nk, 2):
                        r0 = (j2 - nfull) * KB
                        r1 = r0 + KB
                        sp = s_ps.tile([128, 2, QB], F32, tag="s", name="sp")
                        s_matmul(sp[:, 0, r0:QB], pair, h,
                                 slice(j2 * KB, (j2 + 1) * KB), slice(q0 + r0, q0 + QB))
                        s_matmul(sp[:, 1, r1:QB], pair, h,
                                 slice((j2 + 1) * KB, (j2 + 2) * KB), slice(q0 + r1, q0 + QB))
                        pt = ppool.tile([128, 2, QB], P_DT, tag="p", name="pt")
                        if o["exact_diag_exp"]:
                            nc.scalar.activation(out=pt[:, 0, r0:QB], in_=sp[:, 0, r0:QB],
                                                 func=EXP, scale=SCALE)
                            nc.scalar.activation(out=pt[:, 1, r1:QB], in_=sp[:, 1, r1:QB],
                                                 func=EXP, scale=SCALE)
                        else:
                            nc.scalar.activation(
                                out=pt.rearrange("p a b -> p (a b)")[:, r0:2 * QB],
                                in_=sp.rearrange("p a b -> p (a b)")[:, r0:2 * QB],
                                func=EXP, scale=SCALE)
                        nc.gpsimd.tensor_mul(pt[:, 0, r0:r0 + KB], pt[:, 0, r0:r0 + KB], tri_sb[:])
                        nc.gpsimd.tensor_mul(pt[:, 1, r1:r1 + KB], pt[:, 1, r1:r1 + KB], tri_sb[:])
                        ptl.append(pt)

            def emit_pv(pair, h, ptl, onorm):
                    o_p = o_ps.tile([D + 1, QB], F32, tag="o", name="o_t")
                    for j in range(nfull):
                        j2, jj = divmod(j, 2)
                        nc.tensor.matmul(o_p[:],
                                         v_sb[:, j, slot(pair * 2 + h)],
                                         ptl[j2][:, jj, :],
                                         start=(j == 0), stop=False)
                    fill(1)
                    for j in range(nfull, nk):
                        j2, jj = divmod(j, 2)
                        r = (j - nfull) * KB
                        nc.tensor.matmul(o_p[:, r:QB],
                                         v_sb[:, j, slot(pair * 2 + h)],
                                         ptl[j2][:, jj, r:QB],
                                         start=(j == 0 if nfull == 0 else False),
                                         stop=(j == nk - 1))
                    normalize(o_p, onorm, h,
                              splits=(o["norm_splits_last"] if qb == NQB - 1 else 1))

            assert not pv_stream
            if o["pair_interleave"]:
                onorms = [onpool.tile([128, QB], ATTN, tag="onorm", name="onorm_t")
                          for _ in range(2)]
                ptls = {}
                for pair in range(2):
                    for h in range(2):
                        ptls[(pair, h)] = []
                        emit_s(pair, h, ptls[(pair, h)])
                    fill(1)
                for pair in range(2):
                    for h in range(2):
                        emit_pv(pair, h, ptls[(pair, h)], onorms[pair])
                    fill(1)
            else:
                for pair in range(2):
                    onorm = onpool.tile([128, QB], ATTN, tag="onorm", name="onorm_t")
                    ptls = [[], []]
                    for h in range(2):
                        emit_s(pair, h, ptls[h])
                    fill(2)
                    for h in range(2):
                        emit_pv(pair, h, ptls[h], onorm)
                    onorms.append(onorm)


            for u in fill_units:
                outproj_unit(*u)
            for f in fill_fin:
                f()

            # --- output projection: either inline or deferred to the next
            # iteration (after its projections) ---
            if not o["defer_outproj"]:
                emit_outproj(q0, onorms)
            else:
                prev_block = (q0, onorms)

        if o["defer_outproj"] and prev_block is not None:
            emit_outproj(*prev_block, stream_dma=True)

        ctx.close()

    nc.compile()
    return nc


_NC = {}


def _get_program(opts=None):
    key = tuple(sorted((opts or {}).items()))
    if key not in _NC:
        _NC[key] = build_program(opts)
    return _NC[key]


def _make_in_maps(x, Wq, Wk, Wv, Wp, opts=None):
    o = dict(DEFAULT_OPTS)
    if opts:
        o.update(opts)
    pdt = BF_NP if o["proj_bf16"] else np.float32
    adt = BF_NP if o["attn_bf16"] else np.float32
    x = np.asarray(x, dtype=np.float32)
    wqt = np.asarray(Wq, np.float32).T
    wkt = np.asarray(Wk, np.float32).T
    if o["s_fp8"]:
        # reorder per-core GE columns to the d-half-split layout:
        # half-major, then head, then d-within-half
        perm = np.array([hb * 64 + half * 32 + d
                         for half in range(2) for hb in range(4) for d in range(32)])
    else:
        perm = np.arange(GE)
    wvt = np.asarray(Wv, np.float32).T
    wpt = np.asarray(Wp, np.float32).T
    tri = (np.arange(KB)[:, None] <= np.arange(KB)[None, :]).astype(adt)
    ident = np.eye(128, dtype=adt)
    in_maps = []
    for c in range(N_CORES):
        b, hg = c // 4, c % 4
        in_maps.append({
            "xt": np.ascontiguousarray(x[b].T).astype(pdt),
            "wqt": np.ascontiguousarray(wqt[:, hg * GE:(hg + 1) * GE][:, perm]).astype(pdt),
            "wkt": np.ascontiguousarray(wkt[:, hg * GE:(hg + 1) * GE][:, perm]).astype(pdt),
            "wvt": np.ascontiguousarray(wvt[:, hg * GE:(hg + 1) * GE]).astype(pdt),
            "wpt": np.ascontiguousarray(wpt[hg * GE:(hg + 1) * GE, :]).astype(adt),
            "tri": tri,
            "ident": ident,
        })
    return in_maps


def run_cores(x, Wq, Wk, Wv, Wp, bp, **spmd_kwargs):
    """Run the 8-core program; returns (y_full, BassKernelResults)."""
    nc = _get_program()
    in_maps = _make_in_maps(x, Wq, Wk, Wv, Wp)
    res = run_bass_kernel_spmd(nc, in_maps, list(range(N_CORES)), **spmd_kwargs)
    parts = [res.results[c]["y"] for c in range(N_CORES)]
    y = np.empty((B, T, E), np.float32)
    for b in range(B):
        acc = parts[4 * b].astype(np.float32)
        for hg in range(1, 4):
            acc = acc + parts[4 * b + hg].astype(np.float32)
        y[b] = acc
    y += np.asarray(bp, np.float32)[None, None, :]
    return y, res


def kernel(x, Wq, Wk, Wv, Wp, bp):
    y, _ = run_cores(x, Wq, Wk, Wv, Wp, bp)
    return y


def bench(x, Wq, Wk, Wv, Wp, bp, iters=12):
    """Time repeated on-device executions of the compiled program.

    Returns (y_full, list_of_call_seconds). Builds the sharded jit once;
    inputs are device-resident; fresh donated zero outputs are staged
    outside the timed region each iteration.
    """
    import time

    import jax
    import numpy as np_
    from jax.experimental.shard_map import shard_map
    from jax.sharding import Mesh, NamedSharding, PartitionSpec

    from concourse import bass2jax, mybir as mb

    nc = _get_program()
    in_maps = _make_in_maps(x, Wq, Wk, Wv, Wp)
    n_cores = N_CORES
    bass2jax.install_neuronx_cc_hook()

    partition_name = nc.partition_id_tensor.name if nc.partition_id_tensor else None
    in_names, out_names, out_avals, zero_outs = [], [], [], []
    for alloc in nc.m.functions[0].allocations:
        if not isinstance(alloc, mb.MemoryLocationSet):
            continue
        name = alloc.memorylocations[0].name
        if alloc.kind == "ExternalInput":
            if name != partition_name:
                in_names.append(name)
        elif alloc.kind == "ExternalOutput":
            out_names.append(name)
            shape = tuple(alloc.tensor_shape)
            dtype = mb.dt.np(alloc.dtype)
            out_avals.append(jax.core.ShapedArray(shape, dtype))
            zero_outs.append(np_.zeros(shape, dtype))
    n_params = len(in_names)
    all_in_names = in_names + out_names
    if partition_name is not None:
        all_in_names = all_in_names + [partition_name]

    def _body(*args):
        operands = list(args)
        if partition_name is not None:
            operands.append(bass2jax.partition_id_tensor())
        outs = bass2jax._bass_exec_p.bind(
            *operands,
            out_avals=tuple(out_avals),
            in_names=tuple(all_in_names),
            out_names=tuple(out_names),
            lowering_input_output_aliases=(),
            sim_require_finite=True,
            sim_require_nnan=True,
            nc=nc,
        )
        return tuple(outs)

    devices = jax.devices()[:n_cores]
    mesh = Mesh(np_.asarray(devices), ("core",))
    donate = tuple(range(n_params, n_params + len(out_names)))
    sharded = jax.jit(
        shard_map(_body, mesh=mesh,
                  in_specs=(PartitionSpec("core"),) * (n_params + len(out_names)),
                  out_specs=(PartitionSpec("core"),) * len(out_names),
                  check_rep=False),
        donate_argnums=donate, keep_unused=True,
    )
    sh = NamedSharding(mesh, PartitionSpec("core"))
    concat_in = [
        jax.device_put(
            np_.concatenate([np_.asarray(in_maps[c][nm]) for c in range(n_cores)], axis=0), sh)
        for nm in in_names
    ]
    zeros_np = [np_.zeros((n_cores * z.shape[0], *z.shape[1:]), z.dtype) for z in zero_outs]

    times = []
    out_arrs = None
    for it in range(iters):
        dz = [jax.device_put(z, sh) for z in zeros_np]
        jax.block_until_ready(dz)
        t0 = time.perf_counter()
        out_arrs = sharded(*concat_in, *dz)
        jax.block_until_ready(out_arrs)
        times.append(time.perf_counter() - t0)

    parts = [
        np_.asarray(out_arrs[i]).reshape(n_cores, *out_avals[i].shape)
        for i, nm in enumerate(out_names)
    ]
    yi = out_names.index("y")
    y = np_.empty((B, T, E), np_.float32)
    for b in range(B):
        acc = parts[yi][4 * b].astype(np_.float32)
        for hg in range(1, 4):
            acc = acc + parts[yi][4 * b + hg].astype(np_.float32)
        y[b] = acc
    y += np_.asarray(bp, np_.float32)[None, None, :]
    return y, times


# revision 40
# speedup vs baseline: 1.4002x; 1.0967x over previous
"""Multi-head causal attention (B=2, T=2048, E=1024, H=16, D=64) on 8 trn2 cores.

Sharding: core c -> batch b = c // 4, head-group hg = c % 4 (4 heads each).
Per-core: interleaved pipeline over t-chunks — project Q/K/V for chunk t (bf16,
batched DMAs), then causal flash attention for q-block t (S^T[k,q] layout;
Q^T/K^T quantized to fp8e4 in a d-half-split layout so S runs as DoubleRow
matmuls at 0.5 cyc/row; softmax denominator via a ones-augmented V matmul;
P@V and the row-parallel output projection in bf16). Each q-block's output
projection is deferred into the next iteration and interleaved into the
attention stream as stall fillers; the partial [T, E] leaves in bf16 and the
host sums the 4 partials per batch and adds the bias.
"""
import sys
from contextlib import ExitStack

sys.path.insert(0, "/opt/trn_rl_repo")

import numpy as np
import ml_dtypes

import concourse.bass as bass
import concourse.tile as tile
from concourse import bacc, mybir
from concourse.bass_utils import run_bass_kernel_spmd

F32 = mybir.dt.float32
F32R = mybir.dt.float32r
BF = mybir.dt.bfloat16
F8 = mybir.dt.float8e4
BF_NP = ml_dtypes.bfloat16
EXP = mybir.ActivationFunctionType.Exp

B, T, E, H = 2, 2048, 1024, 16
D = E // H              # 64
N_CORES = 8
GH = 4                  # heads per core
GE = GH * D             # 256 per-core projection width
SCALE = float(D) ** -0.5

TCH = 512               # projection t-chunk == attention q-block
NTCH = T // TCH         # 4
KC = 8                  # contraction chunks of 128 over E
QB = 512
NQB = T // QB           # 4
KB = 128                # attention k-block


DEFAULT_OPTS = dict(
    pv_stream=False,    # True: P@V streams V (65-row matmuls per q-tile);
                        # False: P@V streams P (baseline o^T layout)
    proj_bf16=True,     # projection matmul group dtype (x, wq, wk, wv)
    attn_bf16=True,     # attention matmul group dtype (qt, kt, P, v, wp)
    defer_outproj=True,  # emit qb's out-projection after the next chunk's
                         # projections so proj matmuls cover normalize latency
    norm_splits_last=2,
    dma_spread=True,    # issue startup DMAs across SP/DVE/Pool queues
    act_preload=True,   # dummy exp so the act-table load happens at t=0
    s_bufs=2,
    o_bufs=2,
    aux_bufs=2,
    p_bufs=40,
    x_bufs=8,
    on_bufs=4,
    onn_bufs=4,
    l_bufs=8,
    y_bufs=3,
    v_before_k=False,
    xb_bufs=2,
    outproj_fill=True,
    exact_diag_exp=False,
    pair_interleave=True,
    s_fp8=True,
    proj_fp8=2,
)


def build_program(opts=None):
    o = dict(DEFAULT_OPTS)
    if opts:
        o.update(opts)
    pv_stream = o["pv_stream"]
    # Legal dtype groups (HW verifier: matmul operands must share dtype when
    # fp32/f32r is involved):
    #   proj group (x, wq, wk, wv): bf16 (half DMA) or f32r
    #   attn group (qt, kt, P, v, onorm, wp, id): bf16 or f32r
    PROJ = BF if o["proj_bf16"] else F32R
    PROJ_D = BF if o["proj_bf16"] else F32
    ATTN = BF if o["attn_bf16"] else F32R
    ATTN_D = BF if o["attn_bf16"] else F32
    assert o["attn_bf16"] or not pv_stream, "pv_stream needs bf16 V"
    nc = bacc.Bacc("TRN2", target_bir_lowering=False, debug=False, num_devices=N_CORES)

    xt_d = nc.dram_tensor("xt", [E, T], PROJ_D, kind="ExternalInput").ap()
    x8_d = nc.dram_tensor("x8", [E, T], F8, kind="ExternalInput").ap()
    wqt_d = nc.dram_tensor("wqt", [E, GE],
                           F8 if o["proj_fp8"] >= 1 else PROJ_D, kind="ExternalInput").ap()
    wkt_d = nc.dram_tensor("wkt", [E, GE],
                           F8 if o["proj_fp8"] >= 2 else PROJ_D, kind="ExternalInput").ap()
    wvt_d = nc.dram_tensor("wvt", [E, GE], PROJ_D, kind="ExternalInput").ap()
    wpt_d = nc.dram_tensor("wpt", [GE, E], ATTN_D, kind="ExternalInput").ap()
    tri_d = nc.dram_tensor("tri", [KB, KB], ATTN_D, kind="ExternalInput").ap()
    id_d = nc.dram_tensor("ident", [128, 128], ATTN_D, kind="ExternalInput").ap()
    y_d = nc.dram_tensor("y", [T, E], BF, kind="ExternalOutput").ap()

    def slot(hb):
        return slice(hb * (D + 1), (hb + 1) * (D + 1))

    with tile.TileContext(nc) as tc:
        ctx = ExitStack()
        wpool = ctx.enter_context(tc.tile_pool(name="weights", bufs=1))
        qkpool = ctx.enter_context(tc.tile_pool(name="qk", bufs=1))
        vpool = ctx.enter_context(tc.tile_pool(name="vsb", bufs=1))
        xpool = ctx.enter_context(tc.tile_pool(name="xin", bufs=o["x_bufs"]))
        ppool = ctx.enter_context(tc.tile_pool(name="ptile", bufs=o["p_bufs"]))
        onpool = ctx.enter_context(tc.tile_pool(name="onorm", bufs=o["on_bufs"]))
        onnpool = ctx.enter_context(tc.tile_pool(name="on", bufs=o["onn_bufs"]))
        lpool = ctx.enter_context(tc.tile_pool(name="lbc", bufs=o["l_bufs"]))
        ypool = ctx.enter_context(tc.tile_pool(name="ystage", bufs=o["y_bufs"]))
        s_ps = ctx.enter_context(tc.tile_pool(name="s_ps", bufs=o["s_bufs"], space="PSUM"))
        o_ps = ctx.enter_context(tc.tile_pool(name="o_ps", bufs=o["o_bufs"], space="PSUM"))
        aux_ps = ctx.enter_context(tc.tile_pool(name="aux_ps", bufs=o["aux_bufs"], space="PSUM"))

        wq_sb = wpool.tile([128, KC, GE], F8 if o["proj_fp8"] >= 1 else PROJ)
        wk_sb = wpool.tile([128, KC, GE], F8 if o["proj_fp8"] >= 2 else PROJ)
        wv_sb = wpool.tile([128, KC, GE], PROJ)
        wp_sb = wpool.tile([128, 2, E], ATTN)
        tri_sb = wpool.tile([KB, KB], ATTN)
        id_sb = wpool.tile([128, 128], ATTN)

        if o["s_fp8"]:
            # d-half-split layout: partitions hb*32:(hb+1)*32 hold head hb,
            # dim1 is the d-half -- the DoubleRow reduction pair
            qt_sb = qkpool.tile([128, 2, T], F8)
            kt_sb = qkpool.tile([128, 2, T], F8)
        else:
            qt_sb = qkpool.tile([128, 2, T], ATTN)   # pair-stacked Q^T (moving)
            kt_sb = qkpool.tile([128, 2, T], ATTN)   # pair-stacked K^T (stationary)
        v_sb = vpool.tile([128, T // KB, GH * (D + 1)], ATTN)

        if o["act_preload"]:
            # Tiny exp at t=0 so the 1.3us act-table load happens during the
            # startup DMA window, not before the first real softmax.
            warm = wpool.tile([128, 1], F32)
            nc.gpsimd.memset(warm[:], 0.0)
            nc.scalar.activation(out=warm[:], in_=warm[:], func=EXP)

        # ones columns of the augmented V (col D of each 65-wide head slot)
        v_ones = v_sb.rearrange("p b (h c) -> p (b h) c", c=D + 1)[:, :, D:D + 1]
        nc.gpsimd.memset(v_ones, 1.0)

        P_DT = ATTN

        def mload(eng, out_ap, in_ap, r=(PROJ == F32R)):
            eng.dma_start(out=out_ap, in_=in_ap.bitcast(F32R) if r else in_ap)

        def normalize(o_p, onorm, h, splits=1):
            w = QB // splits
            for s in range(splits):
                qs = slice(s * w, (s + 1) * w)
                strip = lpool.tile([1, w], F32, tag="strip", name="strip")
                nc.vector.reciprocal(out=strip[:], in_=o_p[D:D + 1, qs])
                lb = lpool.tile([D, w], F32, tag="lb", name="lb")
                nc.gpsimd.partition_broadcast(lb[:], strip[:])
                nc.vector.tensor_mul(onorm[h * D:(h + 1) * D, qs], o_p[0:D, qs], lb[:])

        def outproj_unit(yt, q0, onorms, qt, nh, stream_dma):
            yp = aux_ps.tile([128, 512], F32, tag="aux", name="yp")
            for pair in range(2):
                nc.tensor.matmul(yp[:],
                                 onorms[pair][:, qt * KB:(qt + 1) * KB],
                                 wp_sb[:, pair, nh * 512:(nh + 1) * 512],
                                 start=(pair == 0), stop=(pair == 1))
            nc.vector.tensor_copy(out=yt[:, qt, nh * 512:(nh + 1) * 512], in_=yp[:])
            if stream_dma and nh == 1:
                nc.sync.dma_start(out=y_d[q0 + qt * KB:q0 + (qt + 1) * KB, :],
                                  in_=yt[:, qt, :])

        def outproj_units(q0, onorms, stream_dma=False):
            yt = ypool.tile([128, QB // KB, E], BF, tag="y", name="yt")
            units = [(yt, q0, onorms, qt, nh, stream_dma)
                     for qt in range(QB // KB) for nh in range(2)]
            fin = []
            if not stream_dma:
                fin.append(lambda: nc.sync.dma_start(
                    out=y_d[q0:q0 + QB, :].rearrange("(a p) n -> p a n", p=128), in_=yt[:]))
            return units, fin

        def emit_outproj(q0, onorms, stream_dma=False):
            units, fin = outproj_units(q0, onorms, stream_dma)
            for u in units:
                outproj_unit(*u)
            for f in fin:
                f()

        prev_block = None  # (q0, onorms) awaiting out-projection
        for tch in range(NTCH):
            ts0 = tch * TCH
            # --- input DMAs (weights ride along with the first t-chunk) ---
            xb = xpool.tile([128, KC, TCH], PROJ, tag="xbig", name="xb", bufs=o["xb_bufs"])
            xr = xt_d.rearrange("(c p) t -> p c t", p=128)
            x8b = None
            if o["proj_fp8"]:
                x8b = xpool.tile([128, KC, TCH], F8, tag="x8big", name="x8b", bufs=o["xb_bufs"])
                x8r = x8_d.rearrange("(c p) t -> p c t", p=128)
            if tch == 0:
                # halved transfers interleaved with weight halves so the first
                # projection matmuls start as soon as possible
                wqr = wqt_d.rearrange("(c p) n -> p c n", p=128)
                wkr = wkt_d.rearrange("(c p) n -> p c n", p=128)
                # (fp8 proj: wq/wk dram tensors already declared F8)
                wvr = wvt_d.rearrange("(c p) n -> p c n", p=128)
                if o["proj_fp8"]:
                    # fp8 Q/K path first: tiny transfers unblock the first
                    # DoubleRow projection almost immediately
                    nc.sync.dma_start(out=x8b[:, 0:4, :], in_=x8r[:, 0:4, ts0:ts0 + TCH])
                    nc.sync.dma_start(out=wq_sb[:], in_=wqr[:])
                    nc.sync.dma_start(out=x8b[:, 4:8, :], in_=x8r[:, 4:8, ts0:ts0 + TCH])
                    nc.sync.dma_start(out=wk_sb[:], in_=wkr[:])
                    mload(nc.sync, xb[:, 0:4, :], xr[:, 0:4, ts0:ts0 + TCH])
                    mload(nc.sync, xb[:, 4:8, :], xr[:, 4:8, ts0:ts0 + TCH])
                else:
                    mload(nc.sync, xb[:, 0:2, :], xr[:, 0:2, ts0:ts0 + TCH])
                    mload(nc.sync, wq_sb[:, 0:2, :], wqr[:, 0:2, :])
                    mload(nc.sync, xb[:, 2:4, :], xr[:, 2:4, ts0:ts0 + TCH])
                    mload(nc.sync, wq_sb[:, 2:4, :], wqr[:, 2:4, :])
                    mload(nc.sync, xb[:, 4:8, :], xr[:, 4:8, ts0:ts0 + TCH])
                    mload(nc.sync, wq_sb[:, 4:8, :], wqr[:, 4:8, :])
                    mload(nc.sync, wk_sb[:, 0:4, :], wkr[:, 0:4, :])
                    mload(nc.sync, wk_sb[:, 4:8, :], wkr[:, 4:8, :])
                weng = nc.gpsimd if o["dma_spread"] else nc.sync
                mload(weng, wv_sb[:, 0:4, :], wvr[:, 0:4, :])
                mload(weng, wv_sb[:, 4:8, :], wvr[:, 4:8, :])
                aload = (ATTN == F32R)
                mload(nc.sync, tri_sb[:], tri_d, r=aload)
                mload(nc.sync, id_sb[:], id_d, r=aload)
                mload(nc.sync, wp_sb[:], wpt_d.rearrange("(c p) n -> p c n", p=128), r=aload)
            else:
                if o["proj_fp8"]:
                    nc.sync.dma_start(out=x8b[:], in_=x8r[:, :, ts0:ts0 + TCH])
                mload(nc.sync, xb[:], xr[:, :, ts0:ts0 + TCH])
            xts = [xb[:, kc, :] for kc in range(KC)]

            # --- Q/K/V projections for this t-chunk ---
            def proj_qk(which, w_sb, t_sb, fp8):
                for pair in range(2):
                    psl = slice(pair * 128, (pair + 1) * 128)
                    pp = aux_ps.tile([128, TCH], F32, tag="aux", name=which)
                    if fp8:
                        DRm = mybir.MatmulPerfMode.DoubleRow
                        for kcp in range(0, KC, 2):
                            nc.tensor.matmul(pp[:], w_sb[:, kcp:kcp + 2, psl],
                                             x8b[:, kcp:kcp + 2, :],
                                             start=(kcp == 0), stop=(kcp == KC - 2),
                                             perf_mode=DRm, tile_position=(0, 0))
                    else:
                        for kc in range(KC):
                            nc.tensor.matmul(pp[:], w_sb[:, kc, psl], xts[kc][:],
                                             start=(kc == 0), stop=(kc == KC - 1))
                    # pair-stacked (bf16 S) or d-half-split fp8 (DoubleRow S):
                    # the host reorders W columns so slot `pair` is the d-half
                    nc.vector.tensor_copy(out=t_sb[:, pair, ts0:ts0 + TCH], in_=pp[:])

            def proj_v():
                for tsub in range(TCH // KB):
                    vp = aux_ps.tile([128, GE], F32, tag="aux", name="vp")
                    for kc in range(KC):
                        nc.tensor.matmul(vp[:], xts[kc][:, tsub * KB:(tsub + 1) * KB],
                                         wv_sb[:, kc, :],
                                         start=(kc == 0), stop=(kc == KC - 1))
                    tb = tch * (TCH // KB) + tsub
                    nc.vector.tensor_copy(
                        out=v_sb[:, tb, :].rearrange("p (h c) -> p h c", c=D + 1)[:, :, 0:D],
                        in_=vp.rearrange("p (h c) -> p h c", c=D),
                    )

            proj_qk("qp", wq_sb, qt_sb, o["proj_fp8"] >= 1)
            if o["v_before_k"]:
                proj_v()
                proj_qk("kp", wk_sb, kt_sb, o["proj_fp8"] >= 2)
            else:
                proj_qk("kp", wk_sb, kt_sb, o["proj_fp8"] >= 2)
                proj_v()

            # deferred out-projection of the previous q-block: either emitted
            # here (proj matmuls cover its normalize latency) or spread into
            # the attention stream as stall fillers
            fill_units, fill_fin = [], []
            if o["defer_outproj"] and prev_block is not None:
                if o["outproj_fill"]:
                    fill_units, fill_fin = outproj_units(*prev_block)
                    fill_units = list(fill_units)
                else:
                    emit_outproj(*prev_block)
                prev_block = None

            def fill(n=1):
                for _ in range(n):
                    if fill_units:
                        outproj_unit(*fill_units.pop(0))

            # --- attention for q-block qb == tch ---
            qb = tch
            q0 = qb * QB
            nk = (q0 + QB) // KB
            nfull = nk - 4
            onorms = []

            DR = mybir.MatmulPerfMode.DoubleRow

            def s_matmul(out_ap, pair, h, jsl, qsl):
                if o["s_fp8"]:
                    hb32 = (pair * 2 + h) * 32
                    hsl = slice(hb32, hb32 + 32)
                    nc.tensor.matmul(out_ap, kt_sb[hsl, :, jsl], qt_sb[hsl, :, qsl],
                                     start=True, stop=True, perf_mode=DR,
                                     tile_position=(hb32, 0))
                else:
                    bsl = slice(h * D, (h + 1) * D)
                    nc.tensor.matmul(out_ap, kt_sb[bsl, pair, jsl], qt_sb[bsl, pair, qsl],
                                     start=True, stop=True)

            def emit_s(pair, h, ptl):
                    for j2 in range(0, nfull, 2):
                        sp = s_ps.tile([128, 2, QB], F32, tag="s", name="sp")
                        for jj in range(2):
                            j = j2 + jj
                            s_matmul(sp[:, jj, :], pair, h,
                                     slice(j * KB, (j + 1) * KB), slice(q0, q0 + QB))
                        pt = ppool.tile([128, 2, QB], P_DT, tag="p", name="pt")
                        nc.scalar.activation(out=pt.rearrange("p a b -> p (a b)"),
                                             in_=sp.rearrange("p a b -> p (a b)"),
                                             func=EXP, scale=SCALE)
                        ptl.append(pt)
                    for j2 in range(nfull, nk, 2):
                        r0 = (j2 - nfull) * KB
                        r1 = r0 + KB
                        sp = s_ps.tile([128, 2, QB], F32, tag="s", name="sp")
                        s_matmul(sp[:, 0, r0:QB], pair, h,
                                 slice(j2 * KB, (j2 + 1) * KB), slice(q0 + r0, q0 + QB))
                        s_matmul(sp[:, 1, r1:QB], pair, h,
                                 slice((j2 + 1) * KB, (j2 + 2) * KB), slice(q0 + r1, q0 + QB))
                        pt = ppool.tile([128, 2, QB], P_DT, tag="p", name="pt")
                        if o["exact_diag_exp"]:
                            nc.scalar.activation(out=pt[:, 0, r0:QB], in_=sp[:, 0, r0:QB],
                                                 func=EXP, scale=SCALE)
                            nc.scalar.activation(out=pt[:, 1, r1:QB], in_=sp[:, 1, r1:QB],
                                                 func=EXP, scale=SCALE)
                        else:
                            nc.scalar.activation(
                                out=pt.rearrange("p a b -> p (a b)")[:, r0:2 * QB],
                                in_=sp.rearrange("p a b -> p (a b)")[:, r0:2 * QB],
                                func=EXP, scale=SCALE)
                        nc.gpsimd.tensor_mul(pt[:, 0, r0:r0 + KB], pt[:, 0, r0:r0 + KB], tri_sb[:])
                        nc.gpsimd.tensor_mul(pt[:, 1, r1:r1 + KB], pt[:, 1, r1:r1 + KB], tri_sb[:])
                        ptl.append(pt)

            def emit_pv(pair, h, ptl, onorm):
                    o_p = o_ps.tile([D + 1, QB], F32, tag="o", name="o_t")
                    for j in range(nfull):
                        j2, jj = divmod(j, 2)
                        nc.tensor.matmul(o_p[:],
                                         v_sb[:, j, slot(pair * 2 + h)],
                                         ptl[j2][:, jj, :],
                                         start=(j == 0), stop=False)
                    fill(1)
                    for j in range(nfull, nk):
                        j2, jj = divmod(j, 2)
                        r = (j - nfull) * KB
                        nc.tensor.matmul(o_p[:, r:QB],
                                         v_sb[:, j, slot(pair * 2 + h)],
                                         ptl[j2][:, jj, r:QB],
                                         start=(j == 0 if nfull == 0 else False),
                                         stop=(j == nk - 1))
                    normalize(o_p, onorm, h,
                              splits=(o["norm_splits_last"] if qb == NQB - 1 else 1))

            assert not pv_stream
            if o["pair_interleave"]:
                onorms = [onpool.tile([128, QB], ATTN, tag="onorm", name="onorm_t")
                          for _ in range(2)]
                ptls = {}
                for pair in range(2):
                    for h in range(2):
                        ptls[(pair, h)] = []
                        emit_s(pair, h, ptls[(pair, h)])
                    fill(1)
                for pair in range(2):
                    for h in range(2):
                        emit_pv(pair, h, ptls[(pair, h)], onorms[pair])
                    fill(1)
            else:
                for pair in range(2):
                    onorm = onpool.tile([128, QB], ATTN, tag="onorm", name="onorm_t")
                    ptls = [[], []]
                    for h in range(2):
                        emit_s(pair, h, ptls[h])
                    fill(2)
                    for h in range(2):
                        emit_pv(pair, h, ptls[h], onorm)
                    onorms.append(onorm)


            for u in fill_units:
                outproj_unit(*u)
            for f in fill_fin:
                f()

            # --- output projection: either inline or deferred to the next
            # iteration (after its projections) ---
            if not o["defer_outproj"]:
                emit_outproj(q0, onorms)
            else:
                prev_block = (q0, onorms)

        if o["defer_outproj"] and prev_block is not None:
            emit_outproj(*prev_block, stream_dma=True)

        ctx.close()

    nc.compile()
    return nc


_NC = {}


def _get_program(opts=None):
    key = tuple(sorted((opts or {}).items()))
    if key not in _NC:
        _NC[key] = build_program(opts)
    return _NC[key]


def _make_in_maps(x, Wq, Wk, Wv, Wp, opts=None):
    o = dict(DEFAULT_OPTS)
    if opts:
        o.update(opts)
    pdt = BF_NP if o["proj_bf16"] else np.float32
    adt = BF_NP if o["attn_bf16"] else np.float32
    x = np.asarray(x, dtype=np.float32)
    wqt = np.asarray(Wq, np.float32).T
    wkt = np.asarray(Wk, np.float32).T
    if o["s_fp8"]:
        # reorder per-core GE columns to the d-half-split layout:
        # half-major, then head, then d-within-half
        perm = np.array([hb * 64 + half * 32 + d
                         for half in range(2) for hb in range(4) for d in range(32)])
    else:
        perm = np.arange(GE)
    wvt = np.asarray(Wv, np.float32).T
    wpt = np.asarray(Wp, np.float32).T
    tri = (np.arange(KB)[:, None] <= np.arange(KB)[None, :]).astype(adt)
    ident = np.eye(128, dtype=adt)
    in_maps = []
    for c in range(N_CORES):
        b, hg = c // 4, c % 4
        f8np = ml_dtypes.float8_e4m3
        in_maps.append({
            "xt": np.ascontiguousarray(x[b].T).astype(pdt),
            "x8": np.ascontiguousarray(x[b].T).astype(f8np),
            "wqt": np.ascontiguousarray(wqt[:, hg * GE:(hg + 1) * GE][:, perm]).astype(
                f8np if o["proj_fp8"] >= 1 else pdt),
            "wkt": np.ascontiguousarray(wkt[:, hg * GE:(hg + 1) * GE][:, perm]).astype(
                f8np if o["proj_fp8"] >= 2 else pdt),
            "wvt": np.ascontiguousarray(wvt[:, hg * GE:(hg + 1) * GE]).astype(pdt),
            "wpt": np.ascontiguousarray(wpt[hg * GE:(hg + 1) * GE, :]).astype(adt),
            "tri": tri,
            "ident": ident,
        })
    return in_maps


def run_cores(x, Wq, Wk, Wv, Wp, bp, **spmd_kwargs):
    """Run the 8-core program; returns (y_full, BassKernelResults)."""
    nc = _get_program()
    in_maps = _make_in_maps(x, Wq, Wk, Wv, Wp)
    res = run_bass_kernel_spmd(nc, in_maps, list(range(N_CORES)), **spmd_kwargs)
    parts = [res.results[c]["y"] for c in range(N_CORES)]
    y = np.empty((B, T, E), np.float32)
    for b in range(B):
        acc = parts[4 * b].astype(np.float32)
        for hg in range(1, 4):
            acc = acc + parts[4 * b + hg].astype(np.float32)
        y[b] = acc
    y += np.asarray(bp, np.float32)[None, None, :]
    return y, res


def kernel(x, Wq, Wk, Wv, Wp, bp):
    y, _ = run_cores(x, Wq, Wk, Wv, Wp, bp)
    return y


def bench(x, Wq, Wk, Wv, Wp, bp, iters=12):
    """Time repeated on-device executions of the compiled program.

    Returns (y_full, list_of_call_seconds). Builds the sharded jit once;
    inputs are device-resident; fresh donated zero outputs are staged
    outside the timed region each iteration.
    """
    import time

    import jax
    import numpy as np_
    from jax.experimental.shard_map import shard_map
    from jax.sharding import Mesh, NamedSharding, PartitionSpec

    from concourse import bass2jax, mybir as mb

    nc = _get_program()
    in_maps = _make_in_maps(x, Wq, Wk, Wv, Wp)
    n_cores = N_CORES
    bass2jax.install_neuronx_cc_hook()

    partition_name = nc.partition_id_tensor.name if nc.partition_id_tensor else None
    in_names, out_names, out_avals, zero_outs = [], [], [], []
    for alloc in nc.m.functions[0].allocations:
        if not isinstance(alloc, mb.MemoryLocationSet):
            continue
        name = alloc.memorylocations[0].name
        if alloc.kind == "ExternalInput":
            if name != partition_name:
                in_names.append(name)
        elif alloc.kind == "ExternalOutput":
            out_names.append(name)
            shape = tuple(alloc.tensor_shape)
            dtype = mb.dt.np(alloc.dtype)
            out_avals.append(jax.core.ShapedArray(shape, dtype))
            zero_outs.append(np_.zeros(shape, dtype))
    n_params = len(in_names)
    all_in_names = in_names + out_names
    if partition_name is not None:
        all_in_names = all_in_names + [partition_name]

    def _body(*args):
        operands = list(args)
        if partition_name is not None:
            operands.append(bass2jax.partition_id_tensor())
        outs = bass2jax._bass_exec_p.bind(
            *operands,
            out_avals=tuple(out_avals),
            in_names=tuple(all_in_names),
            out_names=tuple(out_names),
            lowering_input_output_aliases=(),
            sim_require_finite=True,
            sim_require_nnan=True,
            nc=nc,
        )
        return tuple(outs)

    devices = jax.devices()[:n_cores]
    mesh = Mesh(np_.asarray(devices), ("core",))
    donate = tuple(range(n_params, n_params + len(out_names)))
    sharded = jax.jit(
        shard_map(_body, mesh=mesh,
                  in_specs=(PartitionSpec("core"),) * (n_params + len(out_names)),
                  out_specs=(PartitionSpec("core"),) * len(out_names),
                  check_rep=False),
        donate_argnums=donate, keep_unused=True,
    )
    sh = NamedSharding(mesh, PartitionSpec("core"))
    concat_in = [
        jax.device_put(
            np_.concatenate([np_.asarray(in_maps[c][nm]) for c in range(n_cores)], axis=0), sh)
        for nm in in_names
    ]
    zeros_np = [np_.zeros((n_cores * z.shape[0], *z.shape[1:]), z.dtype) for z in zero_outs]

    times = []
    out_arrs = None
    for it in range(iters):
        dz = [jax.device_put(z, sh) for z in zeros_np]
        jax.block_until_ready(dz)
        t0 = time.perf_counter()
        out_arrs = sharded(*concat_in, *dz)
        jax.block_until_ready(out_arrs)
        times.append(time.perf_counter() - t0)

    parts = [
        np_.asarray(out_arrs[i]).reshape(n_cores, *out_avals[i].shape)
        for i, nm in enumerate(out_names)
    ]
    yi = out_names.index("y")
    y = np_.empty((B, T, E), np_.float32)
    for b in range(B):
        acc = parts[yi][4 * b].astype(np_.float32)
        for hg in range(1, 4):
            acc = acc + parts[yi][4 * b + hg].astype(np_.float32)
        y[b] = acc
    y += np_.asarray(bp, np_.float32)[None, None, :]
    return y, times


# revision 46
# speedup vs baseline: 1.4925x; 1.0660x over previous
"""Multi-head causal attention (B=2, T=2048, E=1024, H=16, D=64) on 8 trn2 cores.

Sharding: core c -> batch b = c // 4, head-group hg = c % 4 (4 heads each).
Per-core: interleaved pipeline over t-chunks — project Q/K/V for chunk t (bf16,
batched DMAs), then causal flash attention for q-block t (S^T[k,q] layout;
Q^T/K^T quantized to fp8e4 in a d-half-split layout so S runs as DoubleRow
matmuls at 0.5 cyc/row; softmax denominator via a ones-augmented V matmul;
P@V and the row-parallel output projection in bf16). Each q-block's output
projection is deferred into the next iteration and interleaved into the
attention stream as stall fillers; the partial [T, E] leaves in bf16 and the
host sums the 4 partials per batch and adds the bias.
"""
import sys
from contextlib import ExitStack

sys.path.insert(0, "/opt/trn_rl_repo")

import numpy as np
import ml_dtypes

import concourse.bass as bass
import concourse.tile as tile
from concourse import bacc, mybir
from concourse.bass_utils import run_bass_kernel_spmd

F32 = mybir.dt.float32
F32R = mybir.dt.float32r
BF = mybir.dt.bfloat16
F8 = mybir.dt.float8e4
BF_NP = ml_dtypes.bfloat16
EXP = mybir.ActivationFunctionType.Exp

B, T, E, H = 2, 2048, 1024, 16
D = E // H              # 64
N_CORES = 8
GH = 4                  # heads per core
GE = GH * D             # 256 per-core projection width
SCALE = float(D) ** -0.5

TCH = 512               # projection t-chunk == attention q-block
NTCH = T // TCH         # 4
KC = 8                  # contraction chunks of 128 over E
QB = 512
NQB = T // QB           # 4
KB = 128                # attention k-block


DEFAULT_OPTS = dict(
    pv_stream=False,    # True: P@V streams V (65-row matmuls per q-tile);
                        # False: P@V streams P (baseline o^T layout)
    proj_bf16=True,     # projection matmul group dtype (x, wq, wk, wv)
    attn_bf16=True,     # attention matmul group dtype (qt, kt, P, v, wp)
    defer_outproj=True,  # emit qb's out-projection after the next chunk's
                         # projections so proj matmuls cover normalize latency
    norm_splits_last=1,
    dma_spread=True,    # issue startup DMAs across SP/DVE/Pool queues
    act_preload=True,   # dummy exp so the act-table load happens at t=0
    s_bufs=2,
    o_bufs=2,
    aux_bufs=2,
    p_bufs=40,
    x_bufs=8,
    on_bufs=4,
    onn_bufs=4,
    l_bufs=8,
    y_bufs=3,
    v_before_k=False,
    xb_bufs=2,
    outproj_fill=True,
    exact_diag_exp=False,
    pair_interleave=True,
    s_fp8=True,
    proj_fp8=1,
    v_late=True,
    k_first_dma=True,
    defer_pv=True,
)


def build_program(opts=None):
    o = dict(DEFAULT_OPTS)
    if opts:
        o.update(opts)
    pv_stream = o["pv_stream"]
    # Legal dtype groups (HW verifier: matmul operands must share dtype when
    # fp32/f32r is involved):
    #   proj group (x, wq, wk, wv): bf16 (half DMA) or f32r
    #   attn group (qt, kt, P, v, onorm, wp, id): bf16 or f32r
    PROJ = BF if o["proj_bf16"] else F32R
    PROJ_D = BF if o["proj_bf16"] else F32
    ATTN = BF if o["attn_bf16"] else F32R
    ATTN_D = BF if o["attn_bf16"] else F32
    assert o["attn_bf16"] or not pv_stream, "pv_stream needs bf16 V"
    nc = bacc.Bacc("TRN2", target_bir_lowering=False, debug=False, num_devices=N_CORES)

    xt_d = nc.dram_tensor("xt", [E, T], PROJ_D, kind="ExternalInput").ap()
    x8_d = nc.dram_tensor("x8", [E, T], F8, kind="ExternalInput").ap()
    wqt_d = nc.dram_tensor("wqt", [E, GE],
                           F8 if o["proj_fp8"] >= 1 else PROJ_D, kind="ExternalInput").ap()
    wkt_d = nc.dram_tensor("wkt", [E, GE],
                           F8 if o["proj_fp8"] >= 2 else PROJ_D, kind="ExternalInput").ap()
    wvt_d = nc.dram_tensor("wvt", [E, GE], PROJ_D, kind="ExternalInput").ap()
    wpt_d = nc.dram_tensor("wpt", [GE, E], ATTN_D, kind="ExternalInput").ap()
    tri_d = nc.dram_tensor("tri", [KB, KB], ATTN_D, kind="ExternalInput").ap()
    id_d = nc.dram_tensor("ident", [128, 128], ATTN_D, kind="ExternalInput").ap()
    y_d = nc.dram_tensor("y", [T, E], BF, kind="ExternalOutput").ap()

    def slot(hb):
        return slice(hb * (D + 1), (hb + 1) * (D + 1))

    with tile.TileContext(nc) as tc:
        ctx = ExitStack()
        wpool = ctx.enter_context(tc.tile_pool(name="weights", bufs=1))
        qkpool = ctx.enter_context(tc.tile_pool(name="qk", bufs=1))
        vpool = ctx.enter_context(tc.tile_pool(name="vsb", bufs=1))
        xpool = ctx.enter_context(tc.tile_pool(name="xin", bufs=o["x_bufs"]))
        ppool = ctx.enter_context(tc.tile_pool(name="ptile", bufs=o["p_bufs"]))
        onpool = ctx.enter_context(tc.tile_pool(name="onorm", bufs=o["on_bufs"]))
        onnpool = ctx.enter_context(tc.tile_pool(name="on", bufs=o["onn_bufs"]))
        lpool = ctx.enter_context(tc.tile_pool(name="lbc", bufs=o["l_bufs"]))
        ypool = ctx.enter_context(tc.tile_pool(name="ystage", bufs=o["y_bufs"]))
        s_ps = ctx.enter_context(tc.tile_pool(name="s_ps", bufs=o["s_bufs"], space="PSUM"))
        o_ps = ctx.enter_context(tc.tile_pool(name="o_ps", bufs=o["o_bufs"], space="PSUM"))
        aux_ps = ctx.enter_context(tc.tile_pool(name="aux_ps", bufs=o["aux_bufs"], space="PSUM"))

        wq_sb = wpool.tile([128, KC, GE], F8 if o["proj_fp8"] >= 1 else PROJ)
        wk_sb = wpool.tile([128, KC, GE], F8 if o["proj_fp8"] >= 2 else PROJ)
        wv_sb = wpool.tile([128, KC, GE], PROJ)
        wp_sb = wpool.tile([128, 2, E], ATTN)
        tri_sb = wpool.tile([KB, KB], ATTN)
        id_sb = wpool.tile([128, 128], ATTN)

        if o["s_fp8"]:
            # d-half-split layout: partitions hb*32:(hb+1)*32 hold head hb,
            # dim1 is the d-half -- the DoubleRow reduction pair
            qt_sb = qkpool.tile([128, 2, T], F8)
            kt_sb = qkpool.tile([128, 2, T], F8)
        else:
            qt_sb = qkpool.tile([128, 2, T], ATTN)   # pair-stacked Q^T (moving)
            kt_sb = qkpool.tile([128, 2, T], ATTN)   # pair-stacked K^T (stationary)
        v_sb = vpool.tile([128, T // KB, GH * (D + 1)], ATTN)

        if o["act_preload"]:
            # Tiny exp at t=0 so the 1.3us act-table load happens during the
            # startup DMA window, not before the first real softmax.
            warm = wpool.tile([128, 1], F32)
            nc.gpsimd.memset(warm[:], 0.0)
            nc.scalar.activation(out=warm[:], in_=warm[:], func=EXP)

        # ones columns of the augmented V (col D of each 65-wide head slot)
        v_ones = v_sb.rearrange("p b (h c) -> p (b h) c", c=D + 1)[:, :, D:D + 1]
        nc.gpsimd.memset(v_ones, 1.0)

        P_DT = ATTN

        def mload(eng, out_ap, in_ap, r=(PROJ == F32R)):
            eng.dma_start(out=out_ap, in_=in_ap.bitcast(F32R) if r else in_ap)

        def normalize(o_p, onorm, h, splits=1):
            w = QB // splits
            for s in range(splits):
                qs = slice(s * w, (s + 1) * w)
                strip = lpool.tile([1, w], F32, tag="strip", name="strip")
                nc.vector.reciprocal(out=strip[:], in_=o_p[D:D + 1, qs])
                lb = lpool.tile([D, w], F32, tag="lb", name="lb")
                nc.gpsimd.partition_broadcast(lb[:], strip[:])
                nc.vector.tensor_mul(onorm[h * D:(h + 1) * D, qs], o_p[0:D, qs], lb[:])

        def outproj_unit(yt, q0, onorms, qt, nh, stream_dma):
            yp = aux_ps.tile([128, 512], F32, tag="aux", name="yp")
            for pair in range(2):
                nc.tensor.matmul(yp[:],
                                 onorms[pair][:, qt * KB:(qt + 1) * KB],
                                 wp_sb[:, pair, nh * 512:(nh + 1) * 512],
                                 start=(pair == 0), stop=(pair == 1))
            nc.vector.tensor_copy(out=yt[:, qt, nh * 512:(nh + 1) * 512], in_=yp[:])
            if stream_dma and nh == 1:
                nc.sync.dma_start(out=y_d[q0 + qt * KB:q0 + (qt + 1) * KB, :],
                                  in_=yt[:, qt, :])

        def outproj_units(q0, onorms, stream_dma=False):
            yt = ypool.tile([128, QB // KB, E], BF, tag="y", name="yt")
            units = [(yt, q0, onorms, qt, nh, stream_dma)
                     for qt in range(QB // KB) for nh in range(2)]
            fin = []
            if not stream_dma:
                fin.append(lambda: nc.sync.dma_start(
                    out=y_d[q0:q0 + QB, :].rearrange("(a p) n -> p a n", p=128), in_=yt[:]))
            return units, fin

        def emit_outproj(q0, onorms, stream_dma=False):
            units, fin = outproj_units(q0, onorms, stream_dma)
            for u in units:
                outproj_unit(*u)
            for f in fin:
                f()

        prev_block = None  # (q0, onorms) awaiting out-projection
        pv_pending = None  # (qb, ptls, emit_pv) awaiting the P@V phase
        run_pv_phase = None
        for tch in range(NTCH):
            ts0 = tch * TCH
            # --- input DMAs (weights ride along with the first t-chunk) ---
            xb = xpool.tile([128, KC, TCH], PROJ, tag="xbig", name="xb", bufs=o["xb_bufs"])
            xr = xt_d.rearrange("(c p) t -> p c t", p=128)
            x8b = None
            if o["proj_fp8"]:
                x8b = xpool.tile([128, KC, TCH], F8, tag="x8big", name="x8b", bufs=o["xb_bufs"])
                x8r = x8_d.rearrange("(c p) t -> p c t", p=128)
            if tch == 0:
                # halved transfers interleaved with weight halves so the first
                # projection matmuls start as soon as possible
                wqr = wqt_d.rearrange("(c p) n -> p c n", p=128)
                wkr = wkt_d.rearrange("(c p) n -> p c n", p=128)
                # (fp8 proj: wq/wk dram tensors already declared F8)
                wvr = wvt_d.rearrange("(c p) n -> p c n", p=128)
                if o["proj_fp8"]:
                    if o["k_first_dma"]:
                        # K path first (widest bf16 transfers gate the first S)
                        mload(nc.sync, xb[:, 0:4, :], xr[:, 0:4, ts0:ts0 + TCH])
                        mload(nc.sync, wk_sb[:, 0:4, :], wkr[:, 0:4, :])
                        mload(nc.sync, xb[:, 4:8, :], xr[:, 4:8, ts0:ts0 + TCH])
                        mload(nc.sync, wk_sb[:, 4:8, :], wkr[:, 4:8, :])
                        nc.sync.dma_start(out=x8b[:, 0:4, :], in_=x8r[:, 0:4, ts0:ts0 + TCH])
                        nc.sync.dma_start(out=wq_sb[:], in_=wqr[:])
                        nc.sync.dma_start(out=x8b[:, 4:8, :], in_=x8r[:, 4:8, ts0:ts0 + TCH])
                    else:
                        nc.sync.dma_start(out=x8b[:, 0:4, :], in_=x8r[:, 0:4, ts0:ts0 + TCH])
                        nc.sync.dma_start(out=wq_sb[:], in_=wqr[:])
                        nc.sync.dma_start(out=x8b[:, 4:8, :], in_=x8r[:, 4:8, ts0:ts0 + TCH])
                        mload(nc.sync, xb[:, 0:4, :], xr[:, 0:4, ts0:ts0 + TCH])
                        mload(nc.sync, wk_sb[:, 0:4, :], wkr[:, 0:4, :])
                        mload(nc.sync, xb[:, 4:8, :], xr[:, 4:8, ts0:ts0 + TCH])
                        mload(nc.sync, wk_sb[:, 4:8, :], wkr[:, 4:8, :])
                else:
                    mload(nc.sync, xb[:, 0:2, :], xr[:, 0:2, ts0:ts0 + TCH])
                    mload(nc.sync, wq_sb[:, 0:2, :], wqr[:, 0:2, :])
                    mload(nc.sync, xb[:, 2:4, :], xr[:, 2:4, ts0:ts0 + TCH])
                    mload(nc.sync, wq_sb[:, 2:4, :], wqr[:, 2:4, :])
                    mload(nc.sync, xb[:, 4:8, :], xr[:, 4:8, ts0:ts0 + TCH])
                    mload(nc.sync, wq_sb[:, 4:8, :], wqr[:, 4:8, :])
                    mload(nc.sync, wk_sb[:, 0:4, :], wkr[:, 0:4, :])
                    mload(nc.sync, wk_sb[:, 4:8, :], wkr[:, 4:8, :])
                weng = nc.gpsimd if o["dma_spread"] else nc.sync
                mload(weng, wv_sb[:, 0:4, :], wvr[:, 0:4, :])
                mload(weng, wv_sb[:, 4:8, :], wvr[:, 4:8, :])
                aload = (ATTN == F32R)
                mload(nc.sync, tri_sb[:], tri_d, r=aload)
                mload(nc.sync, id_sb[:], id_d, r=aload)
                mload(nc.sync, wp_sb[:], wpt_d.rearrange("(c p) n -> p c n", p=128), r=aload)
            else:
                if o["proj_fp8"]:
                    nc.sync.dma_start(out=x8b[:], in_=x8r[:, :, ts0:ts0 + TCH])
                mload(nc.sync, xb[:], xr[:, :, ts0:ts0 + TCH])
            xts = [xb[:, kc, :] for kc in range(KC)]

            # --- Q/K/V projections for this t-chunk ---
            def proj_qk(which, w_sb, t_sb, fp8):
                for pair in range(2):
                    psl = slice(pair * 128, (pair + 1) * 128)
                    pp = aux_ps.tile([128, TCH], F32, tag="aux", name=which)
                    if fp8:
                        DRm = mybir.MatmulPerfMode.DoubleRow
                        for kcp in range(0, KC, 2):
                            nc.tensor.matmul(pp[:], w_sb[:, kcp:kcp + 2, psl],
                                             x8b[:, kcp:kcp + 2, :],
                                             start=(kcp == 0), stop=(kcp == KC - 2),
                                             perf_mode=DRm, tile_position=(0, 0))
                    else:
                        for kc in range(KC):
                            nc.tensor.matmul(pp[:], w_sb[:, kc, psl], xts[kc][:],
                                             start=(kc == 0), stop=(kc == KC - 1))
                    # pair-stacked (bf16 S) or d-half-split fp8 (DoubleRow S):
                    # the host reorders W columns so slot `pair` is the d-half
                    nc.vector.tensor_copy(out=t_sb[:, pair, ts0:ts0 + TCH], in_=pp[:])

            def proj_v():
                for tsub in range(TCH // KB):
                    vp = aux_ps.tile([128, GE], F32, tag="aux", name="vp")
                    for kc in range(KC):
                        nc.tensor.matmul(vp[:], xts[kc][:, tsub * KB:(tsub + 1) * KB],
                                         wv_sb[:, kc, :],
                                         start=(kc == 0), stop=(kc == KC - 1))
                    tb = tch * (TCH // KB) + tsub
                    nc.vector.tensor_copy(
                        out=v_sb[:, tb, :].rearrange("p (h c) -> p h c", c=D + 1)[:, :, 0:D],
                        in_=vp.rearrange("p (h c) -> p h c", c=D),
                    )

            if o["v_late"]:
                # K then Q only; V is emitted mid-attention (after the S
                # phase) so the activation engine gets its exp stream sooner
                proj_qk("kp", wk_sb, kt_sb, o["proj_fp8"] >= 2)
                proj_qk("qp", wq_sb, qt_sb, o["proj_fp8"] >= 1)
            else:
                proj_qk("qp", wq_sb, qt_sb, o["proj_fp8"] >= 1)
                if o["v_before_k"]:
                    proj_v()
                    proj_qk("kp", wk_sb, kt_sb, o["proj_fp8"] >= 2)
                else:
                    proj_qk("kp", wk_sb, kt_sb, o["proj_fp8"] >= 2)
                    proj_v()

            # deferred out-projection of the previous q-block: either emitted
            # here (proj matmuls cover its normalize latency) or spread into
            # the attention stream as stall fillers
            fill_units, fill_fin = [], []
            if o["defer_outproj"] and prev_block is not None:
                if o["outproj_fill"]:
                    fill_units, fill_fin = outproj_units(*prev_block)
                    fill_units = list(fill_units)
                else:
                    emit_outproj(*prev_block)
                prev_block = None

            def fill(n=1):
                for _ in range(n):
                    if fill_units:
                        outproj_unit(*fill_units.pop(0))

            # --- attention: S for q-block qb == tch; P@V optionally deferred
            # one iteration so the next block's S feeds the act engine first ---
            qb = tch
            q0 = qb * QB
            nk = (q0 + QB) // KB
            nfull = nk - 4
            onorms = []

            DR = mybir.MatmulPerfMode.DoubleRow

            def s_matmul(out_ap, pair, h, jsl, qsl):
                if o["s_fp8"]:
                    hb32 = (pair * 2 + h) * 32
                    hsl = slice(hb32, hb32 + 32)
                    nc.tensor.matmul(out_ap, kt_sb[hsl, :, jsl], qt_sb[hsl, :, qsl],
                                     start=True, stop=True, perf_mode=DR,
                                     tile_position=(hb32, 0))
                else:
                    bsl = slice(h * D, (h + 1) * D)
                    nc.tensor.matmul(out_ap, kt_sb[bsl, pair, jsl], qt_sb[bsl, pair, qsl],
                                     start=True, stop=True)

            def emit_s(pair, h, ptl, q0=q0, nk=nk, nfull=nfull):
                    for j2 in range(0, nfull, 2):
                        sp = s_ps.tile([128, 2, QB], F32, tag="s", name="sp")
                        for jj in range(2):
                            j = j2 + jj
                            s_matmul(sp[:, jj, :], pair, h,
                                     slice(j * KB, (j + 1) * KB), slice(q0, q0 + QB))
                        pt = ppool.tile([128, 2, QB], P_DT, tag="p", name="pt")
                        nc.scalar.activation(out=pt.rearrange("p a b -> p (a b)"),
                                             in_=sp.rearrange("p a b -> p (a b)"),
                                             func=EXP, scale=SCALE)
                        ptl.append(pt)
                    for j2 in range(nfull, nk, 2):
                        r0 = (j2 - nfull) * KB
                        r1 = r0 + KB
                        sp = s_ps.tile([128, 2, QB], F32, tag="s", name="sp")
                        s_matmul(sp[:, 0, r0:QB], pair, h,
                                 slice(j2 * KB, (j2 + 1) * KB), slice(q0 + r0, q0 + QB))
                        s_matmul(sp[:, 1, r1:QB], pair, h,
                                 slice((j2 + 1) * KB, (j2 + 2) * KB), slice(q0 + r1, q0 + QB))
                        pt = ppool.tile([128, 2, QB], P_DT, tag="p", name="pt")
                        if o["exact_diag_exp"]:
                            nc.scalar.activation(out=pt[:, 0, r0:QB], in_=sp[:, 0, r0:QB],
                                                 func=EXP, scale=SCALE)
                            nc.scalar.activation(out=pt[:, 1, r1:QB], in_=sp[:, 1, r1:QB],
                                                 func=EXP, scale=SCALE)
                        else:
                            nc.scalar.activation(
                                out=pt.rearrange("p a b -> p (a b)")[:, r0:2 * QB],
                                in_=sp.rearrange("p a b -> p (a b)")[:, r0:2 * QB],
                                func=EXP, scale=SCALE)
                        nc.gpsimd.tensor_mul(pt[:, 0, r0:r0 + KB], pt[:, 0, r0:r0 + KB], tri_sb[:])
                        nc.gpsimd.tensor_mul(pt[:, 1, r1:r1 + KB], pt[:, 1, r1:r1 + KB], tri_sb[:])
                        ptl.append(pt)

            def emit_pv(pair, h, ptl, onorm, pqb=qb, pnk=nk, pnfull=nfull):
                    o_p = o_ps.tile([D + 1, QB], F32, tag="o", name="o_t")
                    for j in range(pnfull):
                        j2, jj = divmod(j, 2)
                        nc.tensor.matmul(o_p[:],
                                         v_sb[:, j, slot(pair * 2 + h)],
                                         ptl[j2][:, jj, :],
                                         start=(j == 0), stop=False)
                    fill(1)
                    for j in range(pnfull, pnk):
                        j2, jj = divmod(j, 2)
                        r = (j - pnfull) * KB
                        nc.tensor.matmul(o_p[:, r:QB],
                                         v_sb[:, j, slot(pair * 2 + h)],
                                         ptl[j2][:, jj, r:QB],
                                         start=(j == 0 if pnfull == 0 else False),
                                         stop=(j == pnk - 1))
                    normalize(o_p, onorm, h,
                              splits=(o["norm_splits_last"] if pqb == NQB - 1 else 1))

            def run_pv_phase(pend):
                pqb, pptls, ppv = pend
                po = [onpool.tile([128, QB], ATTN, tag="onorm", name="onorm_t")
                      for _ in range(2)]
                for pair in range(2):
                    for h in range(2):
                        ppv(pair, h, pptls[(pair, h)], po[pair])
                    fill(1)
                return (pqb * QB, po)

            assert not pv_stream
            if o["pair_interleave"]:
                ptls = {}
                for pair in range(2):
                    for h in range(2):
                        ptls[(pair, h)] = []
                        emit_s(pair, h, ptls[(pair, h)])
                    fill(1)
                if o["v_late"]:
                    proj_v()
                    fill(4)
                if o["defer_pv"]:
                    if pv_pending is not None:
                        prev_block = run_pv_phase(pv_pending)
                    pv_pending = (qb, ptls, emit_pv)
                    for u in fill_units:
                        outproj_unit(*u)
                    for f in fill_fin:
                        f()
                    continue
                onorms = [onpool.tile([128, QB], ATTN, tag="onorm", name="onorm_t")
                          for _ in range(2)]
                for pair in range(2):
                    for h in range(2):
                        emit_pv(pair, h, ptls[(pair, h)], onorms[pair])
                    fill(1)
            else:
                for pair in range(2):
                    onorm = onpool.tile([128, QB], ATTN, tag="onorm", name="onorm_t")
                    ptls = [[], []]
                    for h in range(2):
                        emit_s(pair, h, ptls[h])
                    fill(2)
                    for h in range(2):
                        emit_pv(pair, h, ptls[h], onorm)
                    onorms.append(onorm)


            for u in fill_units:
                outproj_unit(*u)
            for f in fill_fin:
                f()

            # --- output projection: either inline or deferred to the next
            # iteration (after its projections) ---
            if not o["defer_outproj"]:
                emit_outproj(q0, onorms)
            else:
                prev_block = (q0, onorms)

        if o.get("defer_pv") and pv_pending is not None:
            # epilogue: P@V of the last q-block, with the prior block's
            # out-projection as fillers
            fill_units, fill_fin = [], []
            if prev_block is not None:
                fill_units, fill_fin = outproj_units(*prev_block)
                fill_units = list(fill_units)
            prev_block = run_pv_phase(pv_pending)
            for u in fill_units:
                outproj_unit(*u)
            for f in fill_fin:
                f()

        if o["defer_outproj"] and prev_block is not None:
            emit_outproj(*prev_block, stream_dma=True)

        ctx.close()

    nc.compile()
    return nc


_NC = {}


def _get_program(opts=None):
    key = tuple(sorted((opts or {}).items()))
    if key not in _NC:
        _NC[key] = build_program(opts)
    return _NC[key]


def _make_in_maps(x, Wq, Wk, Wv, Wp, opts=None):
    o = dict(DEFAULT_OPTS)
    if opts:
        o.update(opts)
    pdt = BF_NP if o["proj_bf16"] else np.float32
    adt = BF_NP if o["attn_bf16"] else np.float32
    x = np.asarray(x, dtype=np.float32)
    wqt = np.asarray(Wq, np.float32).T
    wkt = np.asarray(Wk, np.float32).T
    if o["s_fp8"]:
        # reorder per-core GE columns to the d-half-split layout:
        # half-major, then head, then d-within-half
        perm = np.array([hb * 64 + half * 32 + d
                         for half in range(2) for hb in range(4) for d in range(32)])
    else:
        perm = np.arange(GE)
    wvt = np.asarray(Wv, np.float32).T
    wpt = np.asarray(Wp, np.float32).T
    tri = (np.arange(KB)[:, None] <= np.arange(KB)[None, :]).astype(adt)
    ident = np.eye(128, dtype=adt)
    in_maps = []
    for c in range(N_CORES):
        b, hg = c // 4, c % 4
        f8np = ml_dtypes.float8_e4m3
        in_maps.append({
            "xt": np.ascontiguousarray(x[b].T).astype(pdt),
            "x8": np.ascontiguousarray(x[b].T).astype(f8np),
            "wqt": np.ascontiguousarray(wqt[:, hg * GE:(hg + 1) * GE][:, perm]).astype(
                f8np if o["proj_fp8"] >= 1 else pdt),
            "wkt": np.ascontiguousarray(wkt[:, hg * GE:(hg + 1) * GE][:, perm]).astype(
                f8np if o["proj_fp8"] >= 2 else pdt),
            "wvt": np.ascontiguousarray(wvt[:, hg * GE:(hg + 1) * GE]).astype(pdt),
            "wpt": np.ascontiguousarray(wpt[hg * GE:(hg + 1) * GE, :]).astype(adt),
            "tri": tri,
            "ident": ident,
        })
    return in_maps


def run_cores(x, Wq, Wk, Wv, Wp, bp, **spmd_kwargs):
    """Run the 8-core program; returns (y_full, BassKernelResults)."""
    nc = _get_program()
    in_maps = _make_in_maps(x, Wq, Wk, Wv, Wp)
    res = run_bass_kernel_spmd(nc, in_maps, list(range(N_CORES)), **spmd_kwargs)
    parts = [res.results[c]["y"] for c in range(N_CORES)]
    y = np.empty((B, T, E), np.float32)
    for b in range(B):
        acc = parts[4 * b].astype(np.float32)
        for hg in range(1, 4):
            acc = acc + parts[4 * b + hg].astype(np.float32)
        y[b] = acc
    y += np.asarray(bp, np.float32)[None, None, :]
    return y, res


def kernel(x, Wq, Wk, Wv, Wp, bp):
    y, _ = run_cores(x, Wq, Wk, Wv, Wp, bp)
    return y


def bench(x, Wq, Wk, Wv, Wp, bp, iters=12):
    """Time repeated on-device executions of the compiled program.

    Returns (y_full, list_of_call_seconds). Builds the sharded jit once;
    inputs are device-resident; fresh donated zero outputs are staged
    outside the timed region each iteration.
    """
    import time

    import jax
    import numpy as np_
    from jax.experimental.shard_map import shard_map
    from jax.sharding import Mesh, NamedSharding, PartitionSpec

    from concourse import bass2jax, mybir as mb

    nc = _get_program()
    in_maps = _make_in_maps(x, Wq, Wk, Wv, Wp)
    n_cores = N_CORES
    bass2jax.install_neuronx_cc_hook()

    partition_name = nc.partition_id_tensor.name if nc.partition_id_tensor else None
    in_names, out_names, out_avals, zero_outs = [], [], [], []
    for alloc in nc.m.functions[0].allocations:
        if not isinstance(alloc, mb.MemoryLocationSet):
            continue
        name = alloc.memorylocations[0].name
        if alloc.kind == "ExternalInput":
            if name != partition_name:
                in_names.append(name)
        elif alloc.kind == "ExternalOutput":
            out_names.append(name)
            shape = tuple(alloc.tensor_shape)
            dtype = mb.dt.np(alloc.dtype)
            out_avals.append(jax.core.ShapedArray(shape, dtype))
            zero_outs.append(np_.zeros(shape, dtype))
    n_params = len(in_names)
    all_in_names = in_names + out_names
    if partition_name is not None:
        all_in_names = all_in_names + [partition_name]

    def _body(*args):
        operands = list(args)
        if partition_name is not None:
            operands.append(bass2jax.partition_id_tensor())
        outs = bass2jax._bass_exec_p.bind(
            *operands,
            out_avals=tuple(out_avals),
            in_names=tuple(all_in_names),
            out_names=tuple(out_names),
            lowering_input_output_aliases=(),
            sim_require_finite=True,
            sim_require_nnan=True,
            nc=nc,
        )
        return tuple(outs)

    devices = jax.devices()[:n_cores]
    mesh = Mesh(np_.asarray(devices), ("core",))
    donate = tuple(range(n_params, n_params + len(out_names)))
    sharded = jax.jit(
        shard_map(_body, mesh=mesh,
                  in_specs=(PartitionSpec("core"),) * (n_params + len(out_names)),
                  out_specs=(PartitionSpec("core"),) * len(out_names),
                  check_rep=False),
        donate_argnums=donate, keep_unused=True,
    )
    sh = NamedSharding(mesh, PartitionSpec("core"))
    concat_in = [
        jax.device_put(
            np_.concatenate([np_.asarray(in_maps[c][nm]) for c in range(n_cores)], axis=0), sh)
        for nm in in_names
    ]
    zeros_np = [np_.zeros((n_cores * z.shape[0], *z.shape[1:]), z.dtype) for z in zero_outs]

    times = []
    out_arrs = None
    for it in range(iters):
        dz = [jax.device_put(z, sh) for z in zeros_np]
        jax.block_until_ready(dz)
        t0 = time.perf_counter()
        out_arrs = sharded(*concat_in, *dz)
        jax.block_until_ready(out_arrs)
        times.append(time.perf_counter() - t0)

    parts = [
        np_.asarray(out_arrs[i]).reshape(n_cores, *out_avals[i].shape)
        for i, nm in enumerate(out_names)
    ]
    yi = out_names.index("y")
    y = np_.empty((B, T, E), np_.float32)
    for b in range(B):
        acc = parts[yi][4 * b].astype(np_.float32)
        for hg in range(1, 4):
            acc = acc + parts[yi][4 * b + hg].astype(np_.float32)
        y[b] = acc
    y += np_.asarray(bp, np_.float32)[None, None, :]
    return y, times


# revision 49
# speedup vs baseline: 1.4946x; 1.0014x over previous
"""Multi-head causal attention (B=2, T=2048, E=1024, H=16, D=64) on 8 trn2 cores.

Sharding: core c -> batch b = c // 4, head-group hg = c % 4 (4 heads each).
Per-core: interleaved pipeline over t-chunks — project Q/K/V for chunk t (bf16,
batched DMAs), then causal flash attention for q-block t (S^T[k,q] layout;
Q^T/K^T quantized to fp8e4 in a d-half-split layout so S runs as DoubleRow
matmuls at 0.5 cyc/row; softmax denominator via a ones-augmented V matmul;
P@V and the row-parallel output projection in bf16). Each q-block's output
projection is deferred into the next iteration and interleaved into the
attention stream as stall fillers; the partial [T, E] leaves in bf16 and the
host sums the 4 partials per batch and adds the bias.
"""
import sys
from contextlib import ExitStack

sys.path.insert(0, "/opt/trn_rl_repo")

import numpy as np
import ml_dtypes

import concourse.bass as bass
import concourse.tile as tile
from concourse import bacc, mybir
from concourse.bass_utils import run_bass_kernel_spmd

F32 = mybir.dt.float32
F32R = mybir.dt.float32r
BF = mybir.dt.bfloat16
F8 = mybir.dt.float8e4
BF_NP = ml_dtypes.bfloat16
EXP = mybir.ActivationFunctionType.Exp

B, T, E, H = 2, 2048, 1024, 16
D = E // H              # 64
N_CORES = 8
GH = 4                  # heads per core
GE = GH * D             # 256 per-core projection width
SCALE = float(D) ** -0.5

TCH = 512               # projection t-chunk == attention q-block
NTCH = T // TCH         # 4
KC = 8                  # contraction chunks of 128 over E
QB = 512
NQB = T // QB           # 4
KB = 128                # attention k-block


DEFAULT_OPTS = dict(
    pv_stream=False,    # True: P@V streams V (65-row matmuls per q-tile);
                        # False: P@V streams P (baseline o^T layout)
    proj_bf16=True,     # projection matmul group dtype (x, wq, wk, wv)
    attn_bf16=True,     # attention matmul group dtype (qt, kt, P, v, wp)
    defer_outproj=True,  # emit qb's out-projection after the next chunk's
                         # projections so proj matmuls cover normalize latency
    norm_splits_last=2,
    dma_spread=True,    # issue startup DMAs across SP/DVE/Pool queues
    act_preload=True,   # dummy exp so the act-table load happens at t=0
    s_bufs=2,
    o_bufs=2,
    aux_bufs=2,
    p_bufs=40,
    x_bufs=8,
    on_bufs=4,
    onn_bufs=4,
    l_bufs=8,
    y_bufs=3,
    v_before_k=False,
    xb_bufs=2,
    outproj_fill=True,
    exact_diag_exp=False,
    pair_interleave=True,
    s_fp8=True,
    proj_fp8=1,
    v_late=True,
    k_first_dma=True,
    defer_pv=True,
    v_defer=False,
    act_tail_copy=True,
)


def build_program(opts=None):
    o = dict(DEFAULT_OPTS)
    if opts:
        o.update(opts)
    pv_stream = o["pv_stream"]
    # Legal dtype groups (HW verifier: matmul operands must share dtype when
    # fp32/f32r is involved):
    #   proj group (x, wq, wk, wv): bf16 (half DMA) or f32r
    #   attn group (qt, kt, P, v, onorm, wp, id): bf16 or f32r
    PROJ = BF if o["proj_bf16"] else F32R
    PROJ_D = BF if o["proj_bf16"] else F32
    ATTN = BF if o["attn_bf16"] else F32R
    ATTN_D = BF if o["attn_bf16"] else F32
    assert o["attn_bf16"] or not pv_stream, "pv_stream needs bf16 V"
    nc = bacc.Bacc("TRN2", target_bir_lowering=False, debug=False, num_devices=N_CORES)

    xt_d = nc.dram_tensor("xt", [E, T], PROJ_D, kind="ExternalInput").ap()
    x8_d = nc.dram_tensor("x8", [E, T], F8, kind="ExternalInput").ap()
    wqt_d = nc.dram_tensor("wqt", [E, GE],
                           F8 if o["proj_fp8"] >= 1 else PROJ_D, kind="ExternalInput").ap()
    wkt_d = nc.dram_tensor("wkt", [E, GE],
                           F8 if o["proj_fp8"] >= 2 else PROJ_D, kind="ExternalInput").ap()
    wvt_d = nc.dram_tensor("wvt", [E, GE], PROJ_D, kind="ExternalInput").ap()
    wpt_d = nc.dram_tensor("wpt", [GE, E], ATTN_D, kind="ExternalInput").ap()
    tri_d = nc.dram_tensor("tri", [KB, KB], ATTN_D, kind="ExternalInput").ap()
    id_d = nc.dram_tensor("ident", [128, 128], ATTN_D, kind="ExternalInput").ap()
    y_d = nc.dram_tensor("y", [T, E], BF, kind="ExternalOutput").ap()

    def slot(hb):
        return slice(hb * (D + 1), (hb + 1) * (D + 1))

    with tile.TileContext(nc) as tc:
        ctx = ExitStack()
        wpool = ctx.enter_context(tc.tile_pool(name="weights", bufs=1))
        qkpool = ctx.enter_context(tc.tile_pool(name="qk", bufs=1))
        vpool = ctx.enter_context(tc.tile_pool(name="vsb", bufs=1))
        xpool = ctx.enter_context(tc.tile_pool(name="xin", bufs=o["x_bufs"]))
        ppool = ctx.enter_context(tc.tile_pool(name="ptile", bufs=o["p_bufs"]))
        onpool = ctx.enter_context(tc.tile_pool(name="onorm", bufs=o["on_bufs"]))
        onnpool = ctx.enter_context(tc.tile_pool(name="on", bufs=o["onn_bufs"]))
        lpool = ctx.enter_context(tc.tile_pool(name="lbc", bufs=o["l_bufs"]))
        ypool = ctx.enter_context(tc.tile_pool(name="ystage", bufs=o["y_bufs"]))
        s_ps = ctx.enter_context(tc.tile_pool(name="s_ps", bufs=o["s_bufs"], space="PSUM"))
        o_ps = ctx.enter_context(tc.tile_pool(name="o_ps", bufs=o["o_bufs"], space="PSUM"))
        aux_ps = ctx.enter_context(tc.tile_pool(name="aux_ps", bufs=o["aux_bufs"], space="PSUM"))

        wq_sb = wpool.tile([128, KC, GE], F8 if o["proj_fp8"] >= 1 else PROJ)
        wk_sb = wpool.tile([128, KC, GE], F8 if o["proj_fp8"] >= 2 else PROJ)
        wv_sb = wpool.tile([128, KC, GE], PROJ)
        wp_sb = wpool.tile([128, 2, E], ATTN)
        tri_sb = wpool.tile([KB, KB], ATTN)
        id_sb = wpool.tile([128, 128], ATTN)

        if o["s_fp8"]:
            # d-half-split layout: partitions hb*32:(hb+1)*32 hold head hb,
            # dim1 is the d-half -- the DoubleRow reduction pair
            qt_sb = qkpool.tile([128, 2, T], F8)
            kt_sb = qkpool.tile([128, 2, T], F8)
        else:
            qt_sb = qkpool.tile([128, 2, T], ATTN)   # pair-stacked Q^T (moving)
            kt_sb = qkpool.tile([128, 2, T], ATTN)   # pair-stacked K^T (stationary)
        v_sb = vpool.tile([128, T // KB, GH * (D + 1)], ATTN)

        if o["act_preload"]:
            # Tiny exp at t=0 so the 1.3us act-table load happens during the
            # startup DMA window, not before the first real softmax.
            warm = wpool.tile([128, 1], F32)
            nc.gpsimd.memset(warm[:], 0.0)
            nc.scalar.activation(out=warm[:], in_=warm[:], func=EXP)

        # ones columns of the augmented V (col D of each 65-wide head slot)
        v_ones = v_sb.rearrange("p b (h c) -> p (b h) c", c=D + 1)[:, :, D:D + 1]
        nc.gpsimd.memset(v_ones, 1.0)

        P_DT = ATTN

        def mload(eng, out_ap, in_ap, r=(PROJ == F32R)):
            eng.dma_start(out=out_ap, in_=in_ap.bitcast(F32R) if r else in_ap)

        def normalize(o_p, onorm, h, splits=1):
            w = QB // splits
            for s in range(splits):
                qs = slice(s * w, (s + 1) * w)
                strip = lpool.tile([1, w], F32, tag="strip", name="strip")
                nc.vector.reciprocal(out=strip[:], in_=o_p[D:D + 1, qs])
                lb = lpool.tile([D, w], F32, tag="lb", name="lb")
                nc.gpsimd.partition_broadcast(lb[:], strip[:])
                nc.vector.tensor_mul(onorm[h * D:(h + 1) * D, qs], o_p[0:D, qs], lb[:])

        def outproj_unit(yt, q0, onorms, qt, nh, stream_dma, act_copy=False):
            yp = aux_ps.tile([128, 512], F32, tag="aux", name="yp")
            for pair in range(2):
                nc.tensor.matmul(yp[:],
                                 onorms[pair][:, qt * KB:(qt + 1) * KB],
                                 wp_sb[:, pair, nh * 512:(nh + 1) * 512],
                                 start=(pair == 0), stop=(pair == 1))
            if act_copy:
                nc.scalar.copy(out=yt[:, qt, nh * 512:(nh + 1) * 512], in_=yp[:])
            else:
                nc.vector.tensor_copy(out=yt[:, qt, nh * 512:(nh + 1) * 512], in_=yp[:])
            if stream_dma and nh == 1:
                nc.sync.dma_start(out=y_d[q0 + qt * KB:q0 + (qt + 1) * KB, :],
                                  in_=yt[:, qt, :])

        def outproj_units(q0, onorms, stream_dma=False):
            yt = ypool.tile([128, QB // KB, E], BF, tag="y", name="yt")
            units = [(yt, q0, onorms, qt, nh, stream_dma)
                     for qt in range(QB // KB) for nh in range(2)]
            fin = []
            if not stream_dma:
                fin.append(lambda: nc.sync.dma_start(
                    out=y_d[q0:q0 + QB, :].rearrange("(a p) n -> p a n", p=128), in_=yt[:]))
            return units, fin

        def emit_outproj(q0, onorms, stream_dma=False):
            units, fin = outproj_units(q0, onorms, stream_dma)
            for u in units:
                outproj_unit(*u)
            for f in fin:
                f()

        prev_block = None  # (q0, onorms) awaiting out-projection
        pv_pending = None  # (qb, ptls, emit_pv) awaiting the P@V phase
        v_pending = None   # (tch, xts) awaiting the deferred V projection
        run_pv_phase = None
        for tch in range(NTCH):
            ts0 = tch * TCH
            # --- input DMAs (weights ride along with the first t-chunk) ---
            xb = xpool.tile([128, KC, TCH], PROJ, tag="xbig", name="xb", bufs=o["xb_bufs"])
            xr = xt_d.rearrange("(c p) t -> p c t", p=128)
            x8b = None
            if o["proj_fp8"]:
                x8b = xpool.tile([128, KC, TCH], F8, tag="x8big", name="x8b", bufs=o["xb_bufs"])
                x8r = x8_d.rearrange("(c p) t -> p c t", p=128)
            if tch == 0:
                # halved transfers interleaved with weight halves so the first
                # projection matmuls start as soon as possible
                wqr = wqt_d.rearrange("(c p) n -> p c n", p=128)
                wkr = wkt_d.rearrange("(c p) n -> p c n", p=128)
                # (fp8 proj: wq/wk dram tensors already declared F8)
                wvr = wvt_d.rearrange("(c p) n -> p c n", p=128)
                if o["proj_fp8"]:
                    if o["k_first_dma"]:
                        # K path first (widest bf16 transfers gate the first S)
                        mload(nc.sync, xb[:, 0:4, :], xr[:, 0:4, ts0:ts0 + TCH])
                        mload(nc.sync, wk_sb[:, 0:4, :], wkr[:, 0:4, :])
                        mload(nc.sync, xb[:, 4:8, :], xr[:, 4:8, ts0:ts0 + TCH])
                        mload(nc.sync, wk_sb[:, 4:8, :], wkr[:, 4:8, :])
                        nc.sync.dma_start(out=x8b[:, 0:4, :], in_=x8r[:, 0:4, ts0:ts0 + TCH])
                        nc.sync.dma_start(out=wq_sb[:], in_=wqr[:])
                        nc.sync.dma_start(out=x8b[:, 4:8, :], in_=x8r[:, 4:8, ts0:ts0 + TCH])
                    else:
                        nc.sync.dma_start(out=x8b[:, 0:4, :], in_=x8r[:, 0:4, ts0:ts0 + TCH])
                        nc.sync.dma_start(out=wq_sb[:], in_=wqr[:])
                        nc.sync.dma_start(out=x8b[:, 4:8, :], in_=x8r[:, 4:8, ts0:ts0 + TCH])
                        mload(nc.sync, xb[:, 0:4, :], xr[:, 0:4, ts0:ts0 + TCH])
                        mload(nc.sync, wk_sb[:, 0:4, :], wkr[:, 0:4, :])
                        mload(nc.sync, xb[:, 4:8, :], xr[:, 4:8, ts0:ts0 + TCH])
                        mload(nc.sync, wk_sb[:, 4:8, :], wkr[:, 4:8, :])
                else:
                    mload(nc.sync, xb[:, 0:2, :], xr[:, 0:2, ts0:ts0 + TCH])
                    mload(nc.sync, wq_sb[:, 0:2, :], wqr[:, 0:2, :])
                    mload(nc.sync, xb[:, 2:4, :], xr[:, 2:4, ts0:ts0 + TCH])
                    mload(nc.sync, wq_sb[:, 2:4, :], wqr[:, 2:4, :])
                    mload(nc.sync, xb[:, 4:8, :], xr[:, 4:8, ts0:ts0 + TCH])
                    mload(nc.sync, wq_sb[:, 4:8, :], wqr[:, 4:8, :])
                    mload(nc.sync, wk_sb[:, 0:4, :], wkr[:, 0:4, :])
                    mload(nc.sync, wk_sb[:, 4:8, :], wkr[:, 4:8, :])
                weng = nc.gpsimd if o["dma_spread"] else nc.sync
                mload(weng, wv_sb[:, 0:4, :], wvr[:, 0:4, :])
                mload(weng, wv_sb[:, 4:8, :], wvr[:, 4:8, :])
                aload = (ATTN == F32R)
                mload(nc.sync, tri_sb[:], tri_d, r=aload)
                mload(nc.sync, id_sb[:], id_d, r=aload)
                mload(nc.sync, wp_sb[:], wpt_d.rearrange("(c p) n -> p c n", p=128), r=aload)
            else:
                if o["proj_fp8"]:
                    nc.sync.dma_start(out=x8b[:], in_=x8r[:, :, ts0:ts0 + TCH])
                mload(nc.sync, xb[:], xr[:, :, ts0:ts0 + TCH])
            xts = [xb[:, kc, :] for kc in range(KC)]

            # --- Q/K/V projections for this t-chunk ---
            def proj_qk(which, w_sb, t_sb, fp8):
                for pair in range(2):
                    psl = slice(pair * 128, (pair + 1) * 128)
                    pp = aux_ps.tile([128, TCH], F32, tag="aux", name=which)
                    if fp8:
                        DRm = mybir.MatmulPerfMode.DoubleRow
                        for kcp in range(0, KC, 2):
                            nc.tensor.matmul(pp[:], w_sb[:, kcp:kcp + 2, psl],
                                             x8b[:, kcp:kcp + 2, :],
                                             start=(kcp == 0), stop=(kcp == KC - 2),
                                             perf_mode=DRm, tile_position=(0, 0))
                    else:
                        for kc in range(KC):
                            nc.tensor.matmul(pp[:], w_sb[:, kc, psl], xts[kc][:],
                                             start=(kc == 0), stop=(kc == KC - 1))
                    # pair-stacked (bf16 S) or d-half-split fp8 (DoubleRow S):
                    # the host reorders W columns so slot `pair` is the d-half
                    nc.vector.tensor_copy(out=t_sb[:, pair, ts0:ts0 + TCH], in_=pp[:])

            def proj_v(vtch=tch, vxts=None):
                vxts = vxts if vxts is not None else xts
                for tsub in range(TCH // KB):
                    vp = aux_ps.tile([128, GE], F32, tag="aux", name="vp")
                    for kc in range(KC):
                        nc.tensor.matmul(vp[:], vxts[kc][:, tsub * KB:(tsub + 1) * KB],
                                         wv_sb[:, kc, :],
                                         start=(kc == 0), stop=(kc == KC - 1))
                    tb = vtch * (TCH // KB) + tsub
                    nc.vector.tensor_copy(
                        out=v_sb[:, tb, :].rearrange("p (h c) -> p h c", c=D + 1)[:, :, 0:D],
                        in_=vp.rearrange("p (h c) -> p h c", c=D),
                    )

            if o["v_late"]:
                # K then Q only; V is emitted mid-attention (after the S
                # phase) so the activation engine gets its exp stream sooner
                proj_qk("kp", wk_sb, kt_sb, o["proj_fp8"] >= 2)
                proj_qk("qp", wq_sb, qt_sb, o["proj_fp8"] >= 1)
            else:
                proj_qk("qp", wq_sb, qt_sb, o["proj_fp8"] >= 1)
                if o["v_before_k"]:
                    proj_v()
                    proj_qk("kp", wk_sb, kt_sb, o["proj_fp8"] >= 2)
                else:
                    proj_qk("kp", wk_sb, kt_sb, o["proj_fp8"] >= 2)
                    proj_v()

            # deferred out-projection of the previous q-block: either emitted
            # here (proj matmuls cover its normalize latency) or spread into
            # the attention stream as stall fillers
            fill_units, fill_fin = [], []
            if o["defer_outproj"] and prev_block is not None:
                if o["outproj_fill"]:
                    fill_units, fill_fin = outproj_units(*prev_block)
                    fill_units = list(fill_units)
                else:
                    emit_outproj(*prev_block)
                prev_block = None

            def fill(n=1):
                for _ in range(n):
                    if fill_units:
                        outproj_unit(*fill_units.pop(0))

            # --- attention: S for q-block qb == tch; P@V optionally deferred
            # one iteration so the next block's S feeds the act engine first ---
            qb = tch
            q0 = qb * QB
            nk = (q0 + QB) // KB
            nfull = nk - 4
            onorms = []

            DR = mybir.MatmulPerfMode.DoubleRow

            def s_matmul(out_ap, pair, h, jsl, qsl):
                if o["s_fp8"]:
                    hb32 = (pair * 2 + h) * 32
                    hsl = slice(hb32, hb32 + 32)
                    nc.tensor.matmul(out_ap, kt_sb[hsl, :, jsl], qt_sb[hsl, :, qsl],
                                     start=True, stop=True, perf_mode=DR,
                                     tile_position=(hb32, 0))
                else:
                    bsl = slice(h * D, (h + 1) * D)
                    nc.tensor.matmul(out_ap, kt_sb[bsl, pair, jsl], qt_sb[bsl, pair, qsl],
                                     start=True, stop=True)

            def emit_s(pair, h, ptl, q0=q0, nk=nk, nfull=nfull):
                    for j2 in range(0, nfull, 2):
                        sp = s_ps.tile([128, 2, QB], F32, tag="s", name="sp")
                        for jj in range(2):
                            j = j2 + jj
                            s_matmul(sp[:, jj, :], pair, h,
                                     slice(j * KB, (j + 1) * KB), slice(q0, q0 + QB))
                        pt = ppool.tile([128, 2, QB], P_DT, tag="p", name="pt")
                        nc.scalar.activation(out=pt.rearrange("p a b -> p (a b)"),
                                             in_=sp.rearrange("p a b -> p (a b)"),
                                             func=EXP, scale=SCALE)
                        ptl.append(pt)
                    for j2 in range(nfull, nk, 2):
                        r0 = (j2 - nfull) * KB
                        r1 = r0 + KB
                        sp = s_ps.tile([128, 2, QB], F32, tag="s", name="sp")
                        s_matmul(sp[:, 0, r0:QB], pair, h,
                                 slice(j2 * KB, (j2 + 1) * KB), slice(q0 + r0, q0 + QB))
                        s_matmul(sp[:, 1, r1:QB], pair, h,
                                 slice((j2 + 1) * KB, (j2 + 2) * KB), slice(q0 + r1, q0 + QB))
                        pt = ppool.tile([128, 2, QB], P_DT, tag="p", name="pt")
                        if o["exact_diag_exp"]:
                            nc.scalar.activation(out=pt[:, 0, r0:QB], in_=sp[:, 0, r0:QB],
                                                 func=EXP, scale=SCALE)
                            nc.scalar.activation(out=pt[:, 1, r1:QB], in_=sp[:, 1, r1:QB],
                                                 func=EXP, scale=SCALE)
                        else:
                            nc.scalar.activation(
                                out=pt.rearrange("p a b -> p (a b)")[:, r0:2 * QB],
                                in_=sp.rearrange("p a b -> p (a b)")[:, r0:2 * QB],
                                func=EXP, scale=SCALE)
                        nc.gpsimd.tensor_mul(pt[:, 0, r0:r0 + KB], pt[:, 0, r0:r0 + KB], tri_sb[:])
                        nc.gpsimd.tensor_mul(pt[:, 1, r1:r1 + KB], pt[:, 1, r1:r1 + KB], tri_sb[:])
                        ptl.append(pt)

            def emit_pv(pair, h, ptl, onorm, pqb=qb, pnk=nk, pnfull=nfull):
                    o_p = o_ps.tile([D + 1, QB], F32, tag="o", name="o_t")
                    for j in range(pnfull):
                        j2, jj = divmod(j, 2)
                        nc.tensor.matmul(o_p[:],
                                         v_sb[:, j, slot(pair * 2 + h)],
                                         ptl[j2][:, jj, :],
                                         start=(j == 0), stop=False)
                    fill(1)
                    for j in range(pnfull, pnk):
                        j2, jj = divmod(j, 2)
                        r = (j - pnfull) * KB
                        nc.tensor.matmul(o_p[:, r:QB],
                                         v_sb[:, j, slot(pair * 2 + h)],
                                         ptl[j2][:, jj, r:QB],
                                         start=(j == 0 if pnfull == 0 else False),
                                         stop=(j == pnk - 1))
                    normalize(o_p, onorm, h,
                              splits=(o["norm_splits_last"] if pqb == NQB - 1 else 1))

            def run_pv_phase(pend):
                pqb, pptls, ppv = pend
                po = [onpool.tile([128, QB], ATTN, tag="onorm", name="onorm_t")
                      for _ in range(2)]
                for pair in range(2):
                    for h in range(2):
                        ppv(pair, h, pptls[(pair, h)], po[pair])
                    fill(1)
                return (pqb * QB, po)

            assert not pv_stream
            if o["pair_interleave"]:
                ptls = {}
                for pair in range(2):
                    for h in range(2):
                        ptls[(pair, h)] = []
                        emit_s(pair, h, ptls[(pair, h)])
                    fill(1)
                if o["v_late"]:
                    if o["v_defer"]:
                        if v_pending is not None:
                            proj_v(*v_pending)
                    else:
                        proj_v()
                    fill(4)
                if o["defer_pv"]:
                    if pv_pending is not None:
                        prev_block = run_pv_phase(pv_pending)
                    pv_pending = (qb, ptls, emit_pv)
                    if o["v_defer"]:
                        v_pending = (tch, xts)
                    for u in fill_units:
                        outproj_unit(*u)
                    for f in fill_fin:
                        f()
                    continue
                onorms = [onpool.tile([128, QB], ATTN, tag="onorm", name="onorm_t")
                          for _ in range(2)]
                for pair in range(2):
                    for h in range(2):
                        emit_pv(pair, h, ptls[(pair, h)], onorms[pair])
                    fill(1)
            else:
                for pair in range(2):
                    onorm = onpool.tile([128, QB], ATTN, tag="onorm", name="onorm_t")
                    ptls = [[], []]
                    for h in range(2):
                        emit_s(pair, h, ptls[h])
                    fill(2)
                    for h in range(2):
                        emit_pv(pair, h, ptls[h], onorm)
                    onorms.append(onorm)


            for u in fill_units:
                outproj_unit(*u)
            for f in fill_fin:
                f()

            # --- output projection: either inline or deferred to the next
            # iteration (after its projections) ---
            if not o["defer_outproj"]:
                emit_outproj(q0, onorms)
            else:
                prev_block = (q0, onorms)

        if o.get("defer_pv") and pv_pending is not None:
            # epilogue: deferred V, then P@V of the last q-block, with the
            # prior block's out-projection as fillers
            if o["v_defer"] and v_pending is not None:
                proj_v(*v_pending)
            fill_units, fill_fin = [], []
            if prev_block is not None:
                fill_units, fill_fin = outproj_units(*prev_block)
                fill_units = list(fill_units)
            prev_block = run_pv_phase(pv_pending)
            for u in fill_units:
                outproj_unit(*u, act_copy=o["act_tail_copy"])
            for f in fill_fin:
                f()

        if o["defer_outproj"] and prev_block is not None:
            q0f, onormsf = prev_block
            ytf = ypool.tile([128, QB // KB, E], BF, tag="y", name="ytf")
            for qt in range(QB // KB):
                for nh in range(2):
                    outproj_unit(ytf, q0f, onormsf, qt, nh, True,
                                 act_copy=o["act_tail_copy"])

        ctx.close()

    nc.compile()
    return nc


_NC = {}


def _get_program(opts=None):
    key = tuple(sorted((opts or {}).items()))
    if key not in _NC:
        _NC[key] = build_program(opts)
    return _NC[key]


def _make_in_maps(x, Wq, Wk, Wv, Wp, opts=None):
    o = dict(DEFAULT_OPTS)
    if opts:
        o.update(opts)
    pdt = BF_NP if o["proj_bf16"] else np.float32
    adt = BF_NP if o["attn_bf16"] else np.float32
    x = np.asarray(x, dtype=np.float32)
    wqt = np.asarray(Wq, np.float32).T
    wkt = np.asarray(Wk, np.float32).T
    if o["s_fp8"]:
        # reorder per-core GE columns to the d-half-split layout:
        # half-major, then head, then d-within-half
        perm = np.array([hb * 64 + half * 32 + d
                         for half in range(2) for hb in range(4) for d in range(32)])
    else:
        perm = np.arange(GE)
    wvt = np.asarray(Wv, np.float32).T
    wpt = np.asarray(Wp, np.float32).T
    tri = (np.arange(KB)[:, None] <= np.arange(KB)[None, :]).astype(adt)
    ident = np.eye(128, dtype=adt)
    in_maps = []
    for c in range(N_CORES):
        b, hg = c // 4, c % 4
        f8np = ml_dtypes.float8_e4m3
        in_maps.append({
            "xt": np.ascontiguousarray(x[b].T).astype(pdt),
            "x8": np.ascontiguousarray(x[b].T).astype(f8np),
            "wqt": np.ascontiguousarray(wqt[:, hg * GE:(hg + 1) * GE][:, perm]).astype(
                f8np if o["proj_fp8"] >= 1 else pdt),
            "wkt": np.ascontiguousarray(wkt[:, hg * GE:(hg + 1) * GE][:, perm]).astype(
                f8np if o["proj_fp8"] >= 2 else pdt),
            "wvt": np.ascontiguousarray(wvt[:, hg * GE:(hg + 1) * GE]).astype(pdt),
            "wpt": np.ascontiguousarray(wpt[hg * GE:(hg + 1) * GE, :]).astype(adt),
            "tri": tri,
            "ident": ident,
        })
    return in_maps


def run_cores(x, Wq, Wk, Wv, Wp, bp, **spmd_kwargs):
    """Run the 8-core program; returns (y_full, BassKernelResults)."""
    nc = _get_program()
    in_maps = _make_in_maps(x, Wq, Wk, Wv, Wp)
    res = run_bass_kernel_spmd(nc, in_maps, list(range(N_CORES)), **spmd_kwargs)
    parts = [res.results[c]["y"] for c in range(N_CORES)]
    y = np.empty((B, T, E), np.float32)
    for b in range(B):
        acc = parts[4 * b].astype(np.float32)
        for hg in range(1, 4):
            acc = acc + parts[4 * b + hg].astype(np.float32)
        y[b] = acc
    y += np.asarray(bp, np.float32)[None, None, :]
    return y, res


def kernel(x, Wq, Wk, Wv, Wp, bp):
    y, _ = run_cores(x, Wq, Wk, Wv, Wp, bp)
    return y


def bench(x, Wq, Wk, Wv, Wp, bp, iters=12):
    """Time repeated on-device executions of the compiled program.

    Returns (y_full, list_of_call_seconds). Builds the sharded jit once;
    inputs are device-resident; fresh donated zero outputs are staged
    outside the timed region each iteration.
    """
    import time

    import jax
    import numpy as np_
    from jax.experimental.shard_map import shard_map
    from jax.sharding import Mesh, NamedSharding, PartitionSpec

    from concourse import bass2jax, mybir as mb

    nc = _get_program()
    in_maps = _make_in_maps(x, Wq, Wk, Wv, Wp)
    n_cores = N_CORES
    bass2jax.install_neuronx_cc_hook()

    partition_name = nc.partition_id_tensor.name if nc.partition_id_tensor else None
    in_names, out_names, out_avals, zero_outs = [], [], [], []
    for alloc in nc.m.functions[0].allocations:
        if not isinstance(alloc, mb.MemoryLocationSet):
            continue
        name = alloc.memorylocations[0].name
        if alloc.kind == "ExternalInput":
            if name != partition_name:
                in_names.append(name)
        elif alloc.kind == "ExternalOutput":
            out_names.append(name)
            shape = tuple(alloc.tensor_shape)
            dtype = mb.dt.np(alloc.dtype)
            out_avals.append(jax.core.ShapedArray(shape, dtype))
            zero_outs.append(np_.zeros(shape, dtype))
    n_params = len(in_names)
    all_in_names = in_names + out_names
    if partition_name is not None:
        all_in_names = all_in_names + [partition_name]

    def _body(*args):
        operands = list(args)
        if partition_name is not None:
            operands.append(bass2jax.partition_id_tensor())
        outs = bass2jax._bass_exec_p.bind(
            *operands,
            out_avals=tuple(out_avals),
            in_names=tuple(all_in_names),
            out_names=tuple(out_names),
            lowering_input_output_aliases=(),
            sim_require_finite=True,
            sim_require_nnan=True,
            nc=nc,
        )
        return tuple(outs)

    devices = jax.devices()[:n_cores]
    mesh = Mesh(np_.asarray(devices), ("core",))
    donate = tuple(range(n_params, n_params + len(out_names)))
    sharded = jax.jit(
        shard_map(_body, mesh=mesh,
                  in_specs=(PartitionSpec("core"),) * (n_params + len(out_names)),
                  out_specs=(PartitionSpec("core"),) * len(out_names),
                  check_rep=False),
        donate_argnums=donate, keep_unused=True,
    )
    sh = NamedSharding(mesh, PartitionSpec("core"))
    concat_in = [
        jax.device_put(
            np_.concatenate([np_.asarray(in_maps[c][nm]) for c in range(n_cores)], axis=0), sh)
        for nm in in_names
    ]
    zeros_np = [np_.zeros((n_cores * z.shape[0], *z.shape[1:]), z.dtype) for z in zero_outs]

    times = []
    out_arrs = None
    for it in range(iters):
        dz = [jax.device_put(z, sh) for z in zeros_np]
        jax.block_until_ready(dz)
        t0 = time.perf_counter()
        out_arrs = sharded(*concat_in, *dz)
        jax.block_until_ready(out_arrs)
        times.append(time.perf_counter() - t0)

    parts = [
        np_.asarray(out_arrs[i]).reshape(n_cores, *out_avals[i].shape)
        for i, nm in enumerate(out_names)
    ]
    yi = out_names.index("y")
    y = np_.empty((B, T, E), np_.float32)
    for b in range(B):
        acc = parts[yi][4 * b].astype(np_.float32)
        for hg in range(1, 4):
            acc = acc + parts[yi][4 * b + hg].astype(np_.float32)
        y[b] = acc
    y += np_.asarray(bp, np_.float32)[None, None, :]
    return y, times
